# revision 5
# baseline (speedup 1.0000x reference)
"""MoE (top-2, capacity-dropped) Trainium2 kernel v2 — expert-parallel, 8 cores.

vs baseline:
- Routing logits sharded per core + AllGather (logits 32KB, xb 2.1MB).
- All-expert capacity cumsum in transposed (tm*8+e, block, token) layout;
  cross-tile offsets via two static-mask matmuls (Msame/Mlow).
- slot->token map built ON-CHIP via one-hot factored matmuls (no DRAM
  scatter/gather staging).
- FFN: weights SBUF-resident, N=512 moving operand, h staged in SBUF bf16,
  W2 per 256-slot half (PSUM: 2 h banks + 4 y banks).
- Combine: y slot-major, AllGathered in 4 chunks overlapped with FFN; home
  cores gather their tokens' <=2 expert rows and combine with weights.
"""

import numpy as np
import ml_dtypes

import concourse.bass as bass
import concourse.tile as tile
from concourse import bacc, mybir
from concourse.bass_utils import run_bass_kernel_spmd
from concourse.masks import make_identity

F32 = mybir.dt.float32
BF16 = mybir.dt.bfloat16
I16 = mybir.dt.int16
AF = mybir.ActivationFunctionType
OP = mybir.AluOpType

P = 128
E = 8
B, S, D = 2, 4096, 1024
H = 4096
T = B * S                  # 8192 tokens
C = 2048                   # capacity per expert
NT = T // P                # 64 token tiles
NTS = NT // E              # 8 token tiles per core slice
TS = T // E                # 1024 tokens per core slice
DC = D // P                # 8 d-chunks
HC = H // P                # 32 h-chunks
NB = 4                     # FFN super-blocks (512 slots each)
SB = C // NB               # 512 slots per super-block
YCH = E * SB               # rows per y AllGather chunk (4096)


def _ge_sum(nc, r2, src, levels, name):
    """acc = sum_k [src >= levels[k]] (compare cascade; all f32)."""
    acc = r2.tile(list(src.shape), F32, name=name)
    tmp = r2.tile(list(src.shape), F32, name=name + "t")
    nc.vector.tensor_scalar(acc[:], src[:], float(levels[0]), None, op0=OP.is_ge)
    for lv in levels[1:]:
        nc.vector.tensor_scalar(tmp[:], src[:], float(lv), None, op0=OP.is_ge)
        nc.vector.tensor_tensor(acc[:], acc[:], tmp[:], op=OP.add)
    return acc


def build_moe(debug=False):
    nc = bacc.Bacc("TRN2", target_bir_lowering=False, debug=False, num_devices=E)

    xs_in = nc.dram_tensor("xs", [TS, D], F32, kind="ExternalInput").ap()
    wg_in = nc.dram_tensor("wg", [P, DC, E], F32, kind="ExternalInput").ap()
    w1_in = nc.dram_tensor("w1s", [P, HC, DC, P], BF16, kind="ExternalInput").ap()
    w2_in = nc.dram_tensor("w2s", [P, HC, D], BF16, kind="ExternalInput").ap()
    b1_in = nc.dram_tensor("b1s", [P, HC], F32, kind="ExternalInput").ap()
    b2_in = nc.dram_tensor("b2r", [1, D], BF16, kind="ExternalInput").ap()
    esel_in = nc.dram_tensor("esel", [P, E], F32, kind="ExternalInput").ap()
    hm64_in = nc.dram_tensor("hm64", [P, NT], F32, kind="ExternalInput").ap()
    erow_in = nc.dram_tensor("erow", [P, E], F32, kind="ExternalInput").ap()
    msame_in = nc.dram_tensor("msame", [P, P], F32, kind="ExternalInput").ap()
    mlow_in = nc.dram_tensor("mlow", [P, P], F32, kind="ExternalInput").ap()
    lo_in = nc.dram_tensor("loall", [P, NT, P], BF16, kind="ExternalInput").ap()
    hi_in = nc.dram_tensor("hiall", [P, NT, 16], F32, kind="ExternalInput").ap()
    thi_in = nc.dram_tensor("thi", [P, NT], F32, kind="ExternalInput").ap()
    tlo_in = nc.dram_tensor("tlo", [P, 1], F32, kind="ExternalInput").ap()

    out_sl = nc.dram_tensor("out_slice", [TS, D], F32, kind="ExternalOutput").ap()

    xb_sl_dram = nc.dram_tensor("xb_slice", [TS, D], BF16)
    xb_all = nc.dram_tensor("xb_all", [T, D], BF16, addr_space="Shared")
    lg_sl_dram = nc.dram_tensor("lg_slice", [E, TS], F32)
    lg_all = nc.dram_tensor("lg_all", [E * E, TS], F32, addr_space="Shared")
    yag_in = nc.dram_tensor("yag_in", [C, D], BF16)
    yag_out = nc.dram_tensor("yag_out", [E * C, D], BF16, addr_space="Shared")
    cc_warm_in = nc.dram_tensor("cc_warm_in", [1, 256], F32)
    cc_warm_out = nc.dram_tensor("cc_warm_out", [E, 256], F32, addr_space="Shared")

    if debug:
        dbg_lg = nc.dram_tensor("dbg_lg", [P, NT, E], F32, kind="ExternalOutput").ap()
        dbg_pos1 = nc.dram_tensor("dbg_pos1", [P, NT], F32, kind="ExternalOutput").ap()
        dbg_pos2 = nc.dram_tensor("dbg_pos2", [P, NT], F32, kind="ExternalOutput").ap()
        dbg_map = nc.dram_tensor("dbg_map", [P, 16], F32, kind="ExternalOutput").ap()
        dbg_gs1 = nc.dram_tensor("dbg_gs1", [P, NT], F32, kind="ExternalOutput").ap()
        dbg_gs2 = nc.dram_tensor("dbg_gs2", [P, NT], F32, kind="ExternalOutput").ap()
        dbg_cw1 = nc.dram_tensor("dbg_cw1", [P, NT], F32, kind="ExternalOutput").ap()
        dbg_cw2 = nc.dram_tensor("dbg_cw2", [P, NT], F32, kind="ExternalOutput").ap()

    with tile.TileContext(nc) as tc:
        with (
            tc.tile_pool(name="const", bufs=1) as const,
            tc.tile_pool(name="persist", bufs=1) as persist,
            tc.tile_pool(name="w1pool", bufs=1) as w1pool,
        ):
            # ---------------- constants ----------------
            ident = const.tile([P, P], F32)
            make_identity(nc, ident[:])
            wg_sb = const.tile([P, DC, E], F32)
            nc.sync.dma_start(wg_sb[:], wg_in[:])
            b1_sb = const.tile([P, HC], F32)
            nc.sync.dma_start(b1_sb[:], b1_in[:])
            b2_sb = const.tile([1, D], BF16)
            nc.sync.dma_start(b2_sb[:], b2_in[:])
            ones1 = const.tile([1, P], BF16)
            nc.vector.memset(ones1[:], 1.0)

            # warm up the collective path so the first real AG avoids the
            # cold-start cost; runs concurrently with P1
            warm_sb = const.tile([1, 256], F32)
            nc.vector.memset(warm_sb[:], 0.0)
            nc.sync.dma_start(cc_warm_in[:], warm_sb[:])
            nc.gpsimd.collective_compute(
                "AllGather", OP.bypass, replica_groups=[list(range(E))],
                ins=[cc_warm_in[:].opt()], outs=[cc_warm_out[:].opt()],
            )

            # W1 resident (hc-major layout) — scalar-queue DMA so the x-slice
            # loads on the sync queue aren't starved behind 8MB of weights
            w1_sb = w1pool.tile([P, HC, DC, P], BF16)
            for hc in range(HC):
                nc.scalar.dma_start(w1_sb[:, hc, :, :], w1_in[:, hc, :, :])

            # ---------------- P1: slice logits + bf16 cast ----------------
            with (
                tc.tile_pool(name="p1x", bufs=1) as p1x,
                tc.tile_pool(name="p1xb", bufs=3) as p1xb,
                tc.tile_pool(name="p1xt", bufs=3) as p1xt,
                tc.tile_pool(name="p1lg", bufs=2) as p1lg,
                tc.tile_pool(name="p1pst", bufs=2, space="PSUM") as p1pst,
                tc.tile_pool(name="p1psl", bufs=2, space="PSUM") as p1psl,
            ):
                # pass A: transposed logits lgT[e, tok] (critical path to AG)
                x_tiles = []
                for i in range(NTS):
                    x_sb = p1x.tile([P, D], F32, name=f"x{i}")
                    nc.sync.dma_start(x_sb[:], xs_in[i * P:(i + 1) * P, :])
                    x_tiles.append(x_sb)
                    lg_ps = p1psl.tile([E, P], F32, space="PSUM")
                    for half in range(2):
                        tr_ps = p1pst.tile([P, 4 * P], F32, space="PSUM")
                        for j in range(4):
                            dc = half * 4 + j
                            nc.tensor.matmul(
                                tr_ps[:, j * P:(j + 1) * P],
                                x_sb[:, dc * P:(dc + 1) * P],
                                ident[:],
                                is_transpose=True,
                                start=(j == 0),
                                stop=(j == 3),
                            )
                        xt_sb = p1xt.tile([P, 4 * P], F32)
                        nc.vector.tensor_copy(xt_sb[:], tr_ps[:])
                        for j in range(4):
                            dc = half * 4 + j
                            nc.tensor.matmul(
                                lg_ps[:],
                                wg_sb[:, dc, :],
                                xt_sb[:, j * P:(j + 1) * P],
                                start=(dc == 0),
                                stop=(dc == DC - 1),
                            )
                    lg_sb = p1lg.tile([E, P], F32)
                    nc.vector.tensor_copy(lg_sb[:], lg_ps[:])
                    nc.sync.dma_start(lg_sl_dram[:, i * P:(i + 1) * P], lg_sb[:])

                nc.gpsimd.collective_compute(
                    "AllGather", OP.bypass, replica_groups=[list(range(E))],
                    ins=[lg_sl_dram[:].opt()], outs=[lg_all[:].opt()],
                )

                # pass B: bf16 cast + staging (xb_all only needed at dispatch)
                for i in range(NTS):
                    xb_sb = p1xb.tile([P, D], BF16)
                    nc.vector.tensor_copy(xb_sb[:], x_tiles[i][:])
                    nc.sync.dma_start(xb_sl_dram[i * P:(i + 1) * P, :], xb_sb[:])

            nc.gpsimd.collective_compute(
                "AllGather", OP.bypass, replica_groups=[list(range(E))],
                ins=[xb_sl_dram[:].opt()], outs=[xb_all[:].opt()],
            )

            # persist tiles used across phases
            cw1_my = persist.tile([P, E], F32)
            cw2_my = persist.tile([P, E], F32)
            idx_h1 = persist.tile([P, TS // 16], I16)   # home gather idx (wrapped)
            idx_h2 = persist.tile([P, TS // 16], I16)
            idx_x = persist.tile([P, C // 16], I16)     # dispatch gather idx

            # ---------------- P2: routing (replicated, from lg_all) ------
            with (
                tc.tile_pool(name="r2", bufs=1) as r2,
                tc.tile_pool(name="ohps", bufs=2, space="PSUM") as ohps,
                tc.tile_pool(name="mmps", bufs=1, space="PSUM") as mmps,
                tc.tile_pool(name="bkps", bufs=1, space="PSUM") as bkps,
                tc.tile_pool(name="mapps", bufs=1, space="PSUM") as mapps,
            ):
                esel_sb = r2.tile([P, E], F32)
                nc.sync.dma_start(esel_sb[:], esel_in[:])
                hm64_sb = r2.tile([P, NT], F32)
                nc.sync.dma_start(hm64_sb[:], hm64_in[:])
                erow_sb = r2.tile([P, E], F32)
                nc.sync.dma_start(erow_sb[:], erow_in[:])
                msame_sb = r2.tile([P, P], F32)
                nc.sync.dma_start(msame_sb[:], msame_in[:])
                mlow_sb = r2.tile([P, P], F32)
                nc.sync.dma_start(mlow_sb[:], mlow_in[:])
                lo_sb = r2.tile([P, NT, P], BF16)
                nc.sync.dma_start(lo_sb[:], lo_in[:])
                hi_sb = r2.tile([P, NT, 16], F32)
                nc.sync.dma_start(hi_sb[:], hi_in[:])
                thi_sb = r2.tile([P, NT], F32)
                nc.sync.dma_start(thi_sb[:], thi_in[:])
                tlo_sb = r2.tile([P, 1], F32)
                nc.sync.dma_start(tlo_sb[:], tlo_in[:])

                lgx = r2.tile([E * E, TS], F32)
                nc.sync.dma_start(lgx[:], lg_all[:])
                lgt = r2.tile([P, NT, E], F32)
                lgt_v = lgt[:].rearrange("p (r j) e -> p r j e", j=NTS)
                lgtr_ps = ohps.tile([P, 4 * P], F32, space="PSUM", name="lgtr")
                for j in range(NTS):
                    nc.tensor.matmul(
                        lgtr_ps[:, j * 64:(j + 1) * 64],
                        lgx[:, j * P:(j + 1) * P],
                        ident[0:E * E, 0:E * E],
                        is_transpose=True,
                        start=(j == 0), stop=(j == NTS - 1),
                    )
                for j in range(NTS):
                    nc.vector.tensor_copy(
                        lgt_v[:, :, j, :],
                        lgtr_ps[:, j * 64:(j + 1) * 64].rearrange("p (r e) -> p r e", e=E),
                    )
                if debug:
                    nc.sync.dma_start(dbg_lg[:], lgt[:])
                # ---- top-2 (token-major) ----
                m1 = r2.tile([P, NT], F32)
                nc.vector.tensor_reduce(m1[:], lgt[:], axis=mybir.AxisListType.X, op=OP.max)
                oh1 = r2.tile([P, NT, E], F32)
                nc.vector.tensor_tensor(
                    oh1[:], lgt[:], m1[:].rearrange("p t -> p t ()").to_broadcast([P, NT, E]),
                    op=OP.is_equal,
                )
                masked = r2.tile([P, NT, E], F32)
                nc.vector.tensor_scalar(masked[:], oh1[:], -1e9, None, op0=OP.mult)
                nc.vector.tensor_tensor(masked[:], masked[:], lgt[:], op=OP.add)
                m2 = r2.tile([P, NT], F32)
                nc.vector.tensor_reduce(m2[:], masked[:], axis=mybir.AxisListType.X, op=OP.max)
                oh2 = r2.tile([P, NT, E], F32)
                nc.vector.tensor_tensor(
                    oh2[:], masked[:], m2[:].rearrange("p t -> p t ()").to_broadcast([P, NT, E]),
                    op=OP.is_equal,
                )
                delta = r2.tile([P, NT], F32)
                nc.vector.tensor_tensor(delta[:], m2[:], m1[:], op=OP.subtract)
                wr1 = r2.tile([P, NT], F32)
                nc.scalar.activation(wr1[:], delta[:], AF.Sigmoid, scale=-1.0)
                wr2 = r2.tile([P, NT], F32)
                nc.scalar.activation(wr2[:], delta[:], AF.Sigmoid)

                # ---- all-expert capacity cumsum, P2 layout (tm*8+e, b, tok) ----
                ohs = [oh1, oh2]
                csm1T = []      # token-major (cs-1)*keep*oh per rank  [P, NT, E]
                kT = []         # token-major keep*oh per rank         [P, NT, E]
                base1 = None
                for r in range(2):
                    ohT_ps = ohps.tile([P, 4 * P], F32, space="PSUM", name="ohT")
                    ohsv = ohs[r][:].rearrange("p a e -> p (a e)")
                    for b in range(4):
                        nc.tensor.matmul(
                            ohT_ps[:, b * P:(b + 1) * P],
                            ohsv[:, b * P:(b + 1) * P],
                            ident[:],
                            is_transpose=True,
                            start=(b == 0), stop=(b == 3),
                        )
                    ohT = r2.tile([P, 4, P], F32, name=f"ohTs{r}")
                    nc.vector.tensor_copy(ohT[:], ohT_ps[:].rearrange("p (b t) -> p b t", b=4))
                    ic = r2.tile([P, 4, P], F32, name=f"ic{r}")
                    for b in range(4):
                        nc.vector.tensor_tensor_scan(
                            ic[:, b, :], ohT[:, b, :], ohT[:, b, :], 0.0,
                            op0=OP.add, op1=OP.bypass,
                        )
                    cnt = r2.tile([P, 4], F32, name=f"cnt{r}")
                    nc.vector.tensor_copy(cnt[:], ic[:, :, P - 1])
                    # cross-tile offsets: same-expert block totals + intra lower
                    mm_ps = mmps.tile([P, 8], F32, space="PSUM", name="mm")
                    nc.tensor.matmul(mm_ps[:, 0:4], msame_sb[:], cnt[:], start=True, stop=False)
                    nc.tensor.matmul(mm_ps[:, 4:8], mlow_sb[:], cnt[:], start=False, stop=True)
                    mm_sb = r2.tile([P, 8], F32, name=f"mmsb{r}")
                    nc.vector.tensor_copy(mm_sb[:], mm_ps[:])
                    btot_i = r2.tile([P, 4], F32, name=f"bti{r}")
                    nc.vector.tensor_tensor_scan(
                        btot_i[:], mm_sb[:, 0:4], mm_sb[:, 0:4], 0.0,
                        op0=OP.add, op1=OP.bypass,
                    )
                    offs = r2.tile([P, 4], F32, name=f"offs{r}")
                    nc.vector.tensor_tensor(offs[:], btot_i[:], mm_sb[:, 0:4], op=OP.subtract)
                    nc.vector.tensor_tensor(offs[:], offs[:], mm_sb[:, 4:8], op=OP.add)
                    if r == 1:
                        nc.vector.tensor_scalar(offs[:], offs[:], base1[:], None, op0=OP.add)
                    cs = r2.tile([P, 4, P], F32, name=f"cs{r}")
                    for b in range(4):
                        nc.vector.tensor_scalar(
                            cs[:, b, :], ic[:, b, :], offs[:, b:b + 1], None, op0=OP.add
                        )
                    if r == 0:
                        # rank-1 base: min(total rank-0 assigned per expert, C);
                        # mm_sb[:, 0:4] holds per-expert block totals (Msame).
                        n0 = r2.tile([P, 1], F32)
                        nc.vector.tensor_reduce(n0[:], mm_sb[:, 0:4], axis=mybir.AxisListType.X, op=OP.add)
                        base1 = r2.tile([P, 1], F32)
                        nc.vector.tensor_scalar(base1[:], n0[:], float(C), None, op0=OP.min)
                    keep = r2.tile([P, 4, P], F32, name=f"keep{r}")
                    nc.vector.tensor_scalar(keep[:], cs[:], float(C), None, op0=OP.is_le)
                    kk = r2.tile([P, 4, P], F32, name=f"kk{r}")
                    nc.vector.tensor_tensor(kk[:], keep[:], ohT[:], op=OP.mult)
                    ksl = r2.tile([P, 4, P], F32, name=f"ksl{r}")
                    nc.vector.tensor_scalar(ksl[:], cs[:], -1.0, None, op0=OP.add)
                    nc.vector.tensor_tensor(ksl[:], ksl[:], kk[:], op=OP.mult)
                    # transpose back to token-major (two 1-bank psum tiles)
                    bk1 = bkps.tile([P, 4 * P], F32, space="PSUM", name="bk1")
                    bk2 = bkps.tile([P, 4 * P], F32, space="PSUM", name="bk2")
                    for b in range(4):
                        nc.tensor.matmul(
                            bk1[:, b * P:(b + 1) * P], ksl[:, b, :], ident[:],
                            is_transpose=True, start=(b == 0), stop=(b == 3),
                        )
                    for b in range(4):
                        nc.tensor.matmul(
                            bk2[:, b * P:(b + 1) * P], kk[:, b, :], ident[:],
                            is_transpose=True, start=(b == 0), stop=(b == 3),
                        )
                    cT = r2.tile([P, NT, E], F32, name=f"cT{r}")
                    nc.vector.tensor_copy(cT[:], bk1[:].rearrange("p (a e) -> p a e", e=E))
                    kTr = r2.tile([P, NT, E], F32, name=f"kTr{r}")
                    nc.vector.tensor_copy(kTr[:], bk2[:].rearrange("p (a e) -> p a e", e=E))
                    csm1T.append(cT)
                    kT.append(kTr)

                # ---- home-side indices (token-major, all tiles) ----
                esel_b = esel_sb[:].rearrange("p e -> p () e").to_broadcast([P, NT, E])
                erow_b = erow_sb[:].rearrange("p e -> p () e").to_broadcast([P, NT, E])
                tmp3 = r2.tile([P, NT, E], F32)
                gs = []
                cwf = []
                for r in range(2):
                    pos = r2.tile([P, NT], F32, name=f"pos{r}")
                    nc.vector.tensor_reduce(pos[:], csm1T[r][:], axis=mybir.AxisListType.X, op=OP.add)
                    keep_s = r2.tile([P, NT], F32, name=f"ks{r}")
                    nc.vector.tensor_reduce(keep_s[:], kT[r][:], axis=mybir.AxisListType.X, op=OP.max)
                    nc.vector.tensor_tensor(tmp3[:], ohs[r][:], erow_b, op=OP.mult)
                    es = r2.tile([P, NT], F32, name=f"es{r}")
                    nc.vector.tensor_reduce(es[:], tmp3[:], axis=mybir.AxisListType.X, op=OP.max)
                    # AG row index: 2048*(s//256) + 256*e + s%256
                    q = _ge_sum(nc, r2, pos, [256.0 * k for k in range(1, 8)], f"q{r}")
                    g = r2.tile([P, NT], F32, name=f"g{r}")
                    rem = r2.tile([P, NT], F32, name=f"rm{r}")
                    nc.vector.tensor_scalar(rem[:], q[:], -256.0, None, op0=OP.mult)
                    nc.vector.tensor_tensor(rem[:], rem[:], pos[:], op=OP.add)
                    nc.vector.tensor_scalar(g[:], q[:], 2048.0, None, op0=OP.mult)
                    t2 = r2.tile([P, NT], F32, name=f"t2{r}")
                    nc.vector.tensor_scalar(t2[:], es[:], 256.0, None, op0=OP.mult)
                    nc.vector.tensor_tensor(g[:], g[:], t2[:], op=OP.add)
                    nc.vector.tensor_tensor(g[:], g[:], rem[:], op=OP.add)
                    cw = r2.tile([P, NT], F32, name=f"cw{r}")
                    wsrc = wr1 if r == 0 else wr2
                    nc.vector.tensor_tensor(cw[:], wsrc[:], keep_s[:], op=OP.mult)
                    gs.append(g)
                    cwf.append(cw)
                    if debug:
                        nc.sync.dma_start([dbg_gs1, dbg_gs2][r][:], g[:])
                        nc.sync.dma_start([dbg_cw1, dbg_cw2][r][:], cw[:])
                        nc.sync.dma_start([dbg_pos1, dbg_pos2][r][:], pos[:])

                # select MY home block (hm64 mask + log-fold), build wrapped idx
                for r in range(2):
                    msk_g = r2.tile([P, NT], F32, name=f"mg{r}")
                    nc.vector.tensor_tensor(msk_g[:], gs[r][:], hm64_sb[:], op=OP.mult)
                    msk_c = r2.tile([P, NT], F32, name=f"mc{r}")
                    nc.vector.tensor_tensor(msk_c[:], cwf[r][:], hm64_sb[:], op=OP.mult)
                    for half in (32, 16, 8):
                        nc.vector.tensor_tensor(
                            msk_g[:, 0:half], msk_g[:, 0:half], msk_g[:, half:2 * half], op=OP.add
                        )
                        nc.vector.tensor_tensor(
                            msk_c[:, 0:half], msk_c[:, 0:half], msk_c[:, half:2 * half], op=OP.add
                        )
                    nc.vector.tensor_copy([cw1_my, cw2_my][r][:], msk_c[:, 0:E])
                    gi = r2.tile([P, E], I16, name=f"gi{r}")
                    nc.vector.tensor_copy(gi[:], msk_g[:, 0:E])
                    sh16 = [(i + 16) % 32 for i in range(32)]
                    gish = r2.tile([P, E], I16, name=f"gish{r}")
                    nc.vector.stream_shuffle(gish[:], gi[:], sh16)
                    idxh = [idx_h1, idx_h2][r]
                    idxh_v = idxh[0:16, :].rearrange("r (c q) -> r c q", q=8)
                    for qq in range(8):
                        src = gi if qq % 2 == 0 else gish
                        nc.vector.tensor_copy(
                            idxh_v[:, :, qq],
                            src[(qq // 2) * 32:(qq // 2) * 32 + 16, :],
                        )
                    for k in range(1, 8):
                        nc.sync.dma_start(idxh[16 * k:16 * (k + 1), :], idxh[0:16, :])

                # ---- expert-side slot->token map (my expert) ----
                ksl_e = r2.tile([P, NT], F32)
                k_e = r2.tile([P, NT], F32)
                acc = r2.tile([P, NT], F32)
                for r in range(2):
                    nc.vector.tensor_tensor(tmp3[:], csm1T[r][:], esel_b, op=OP.mult)
                    nc.vector.tensor_reduce(
                        (acc if r else ksl_e)[:], tmp3[:], axis=mybir.AxisListType.X, op=OP.add
                    )
                    if r:
                        nc.vector.tensor_tensor(ksl_e[:], ksl_e[:], acc[:], op=OP.add)
                    nc.vector.tensor_tensor(tmp3[:], kT[r][:], esel_b, op=OP.mult)
                    nc.vector.tensor_reduce(
                        (acc if r else k_e)[:], tmp3[:], axis=mybir.AxisListType.X, op=OP.max
                    )
                    if r:
                        nc.vector.tensor_tensor(k_e[:], k_e[:], acc[:], op=OP.max)
                # chi = s//128 in [0,16), remc = s%128 via two-level cascade
                q8 = _ge_sum(nc, r2, ksl_e, [512.0, 1024.0, 1536.0], "q8")
                s1 = r2.tile([P, NT], F32)
                nc.vector.tensor_scalar(s1[:], q8[:], -512.0, None, op0=OP.mult)
                nc.vector.tensor_tensor(s1[:], s1[:], ksl_e[:], op=OP.add)
                c3 = _ge_sum(nc, r2, s1, [128.0, 256.0, 384.0], "c3")
                chi = r2.tile([P, NT], F32)
                nc.vector.tensor_scalar(chi[:], q8[:], 4.0, None, op0=OP.mult)
                nc.vector.tensor_tensor(chi[:], chi[:], c3[:], op=OP.add)
                remc = r2.tile([P, NT], F32)
                nc.vector.tensor_scalar(remc[:], c3[:], -128.0, None, op0=OP.mult)
                nc.vector.tensor_tensor(remc[:], remc[:], s1[:], op=OP.add)
                ktlo = r2.tile([P, NT], F32)
                nc.vector.tensor_scalar(ktlo[:], k_e[:], tlo_sb[:], None, op0=OP.mult)
                kthi = r2.tile([P, NT], F32)
                nc.vector.tensor_tensor(kthi[:], k_e[:], thi_sb[:], op=OP.mult)

                o_all = r2.tile([P, NT, P], BF16)   # [s%128 == lo]
                nc.vector.tensor_tensor(
                    o_all[:], lo_sb[:],
                    remc[:].rearrange("p t -> p t ()").to_broadcast([P, NT, P]),
                    op=OP.is_equal,
                )
                v0 = r2.tile([P, NT, 16], F32)
                nc.vector.tensor_tensor(
                    v0[:], hi_sb[:],
                    chi[:].rearrange("p t -> p t ()").to_broadcast([P, NT, 16]),
                    op=OP.is_equal,
                )
                v_all = r2.tile([P, NT, 2, 16], BF16)
                nc.vector.tensor_tensor(
                    v_all[:, :, 0, :], v0[:],
                    ktlo[:].rearrange("p t -> p t ()").to_broadcast([P, NT, 16]),
                    op=OP.mult,
                )
                nc.vector.tensor_tensor(
                    v_all[:, :, 1, :], v0[:],
                    kthi[:].rearrange("p t -> p t ()").to_broadcast([P, NT, 16]),
                    op=OP.mult,
                )
                map_ps = mapps.tile([P, 2, 16], F32, space="PSUM")
                for j in range(NT):
                    nc.tensor.matmul(
                        map_ps[:].rearrange("p a b -> p (a b)"),
                        o_all[:, j, :],
                        v_all[:, j, :, :].rearrange("p a b -> p (a b)"),
                        start=(j == 0), stop=(j == NT - 1),
                    )
                map_sb = r2.tile([P, 2, 16], F32)
                nc.vector.tensor_copy(map_sb[:], map_ps[:])
                map_tok = r2.tile([P, 16], F32)
                nc.vector.tensor_scalar(map_tok[:], map_sb[:, 1, :], 64.0, None, op0=OP.mult)
                nc.vector.tensor_tensor(map_tok[:], map_tok[:], map_sb[:, 0, :], op=OP.add)
                if debug:
                    nc.sync.dma_start(dbg_map[:], map_tok[:])
                mi = r2.tile([P, 16], I16)
                nc.vector.tensor_copy(mi[:], map_tok[:])
                sh16 = [(i + 16) % 32 for i in range(32)]
                mish = r2.tile([P, 16], I16)
                nc.vector.stream_shuffle(mish[:], mi[:], sh16)
                idxx_v = idx_x[0:16, :].rearrange("r (h q) -> r h q", q=8)
                for qq in range(8):
                    src = mi if qq % 2 == 0 else mish
                    nc.vector.tensor_copy(
                        idxx_v[:, :, qq],
                        src[(qq // 2) * 32:(qq // 2) * 32 + 16, :],
                    )
                for k in range(1, 8):
                    nc.sync.dma_start(idx_x[16 * k:16 * (k + 1), :], idx_x[0:16, :])

            # ---------------- P3: FFN ----------------
            with (
                tc.tile_pool(name="w2pool", bufs=1) as w2pool,
                tc.tile_pool(name="xte", bufs=1) as xtep,
                tc.tile_pool(name="hall", bufs=1) as hallp,
                tc.tile_pool(name="ypool", bufs=1) as ypool,
                tc.tile_pool(name="hps", bufs=2, space="PSUM") as hps,
                tc.tile_pool(name="yps", bufs=1, space="PSUM") as yps,
            ):
                w2_sb = w2pool.tile([P, HC, D], BF16)
                for hc in range(HC):
                    nc.sync.dma_start(w2_sb[:, hc, :], w2_in[:, hc, :])
                h_all = hallp.tile([P, HC, SB], BF16)
                # all dispatch gathers up-front: gpsimd queue is in-order and
                # collective triggers block it, so gathers must precede them
                xTes = []
                for sb in range(NB):
                    xTe = xtep.tile([P, DC, SB], BF16, name=f"xTe{sb}")
                    nc.gpsimd.dma_gather(
                        out_ap=xTe[:],
                        in_ap=xb_all[:],
                        idxs_ap=idx_x[:, sb * (SB // 16):(sb + 1) * (SB // 16)],
                        num_idxs=SB, num_idxs_reg=SB, elem_size=D, transpose=True,
                    )
                    xTes.append(xTe)
                for sb in range(NB):
                    xTe = xTes[sb]
                    for hc in range(HC):
                        h_ps = hps.tile([P, SB], F32, space="PSUM", name="hps")
                        for dc in range(DC):
                            nc.tensor.matmul(
                                h_ps[:],
                                w1_sb[:, hc, dc, :],
                                xTe[:, dc, :],
                                start=(dc == 0), stop=(dc == DC - 1),
                            )
                        nc.scalar.activation(
                            h_all[:, hc, :], h_ps[:], AF.Gelu_apprx_tanh,
                            bias=b1_sb[:, hc:hc + 1],
                        )
                    for half in range(2):
                        y_ts = [
                            [yps.tile([P, 512], F32, space="PSUM", name=f"y{st}{dg}") for dg in range(2)]
                            for st in range(2)
                        ]
                        for hc in range(HC):
                            for st in range(2):
                                so = half * 256 + st * P
                                for dg in range(2):
                                    nc.tensor.matmul(
                                        y_ts[st][dg][:],
                                        h_all[:, hc, so:so + P],
                                        w2_sb[:, hc, dg * 512:(dg + 1) * 512],
                                        start=(hc == 0), stop=False,
                                    )
                        y_sb = ypool.tile([P, 2, D], BF16, name="ysb")
                        for st in range(2):
                            for dg in range(2):
                                nc.tensor.matmul(
                                    y_ts[st][dg][:], ones1[:],
                                    b2_sb[:, dg * 512:(dg + 1) * 512],
                                    start=False, stop=True,
                                )
                                nc.scalar.activation(
                                    y_sb[:, st, dg * 512:(dg + 1) * 512],
                                    y_ts[st][dg][:], AF.Copy,
                                )
                        r0 = sb * SB + half * 256
                        nc.sync.dma_start(
                            yag_in[r0:r0 + 256, :].rearrange("(s p) d -> p s d", p=P),
                            y_sb[:],
                        )
                        ch = 2 * sb + half
                        nc.gpsimd.collective_compute(
                            "AllGather", OP.bypass, replica_groups=[list(range(E))],
                            ins=[yag_in[ch * 256:(ch + 1) * 256, :].opt()],
                            outs=[yag_out[ch * 2048:(ch + 1) * 2048, :].opt()],
                        )

            # ---------------- P4: home combine (2 pipelined halves) -------
            NH = NTS // 2
            with tc.tile_pool(name="homep", bufs=2) as homep:
                for hh in range(2):
                    c0 = hh * NH
                    g1 = homep.tile([P, NH, D], BF16, name="g1")
                    nc.gpsimd.dma_gather(
                        out_ap=g1[:], in_ap=yag_out[:],
                        idxs_ap=idx_h1[:, c0 * 8:(c0 + NH) * 8],
                        num_idxs=NH * P, num_idxs_reg=NH * P, elem_size=D,
                    )
                    g2 = homep.tile([P, NH, D], BF16, name="g2")
                    nc.gpsimd.dma_gather(
                        out_ap=g2[:], in_ap=yag_out[:],
                        idxs_ap=idx_h2[:, c0 * 8:(c0 + NH) * 8],
                        num_idxs=NH * P, num_idxs_reg=NH * P, elem_size=D,
                    )
                    o1 = homep.tile([P, NH, D], F32, name="o1")
                    nc.vector.tensor_tensor(
                        o1[:], g1[:],
                        cw1_my[:, c0:c0 + NH].rearrange("p c -> p c ()").to_broadcast([P, NH, D]),
                        op=OP.mult,
                    )
                    o2 = homep.tile([P, NH, D], F32, name="o2")
                    nc.vector.tensor_tensor(
                        o2[:], g2[:],
                        cw2_my[:, c0:c0 + NH].rearrange("p c -> p c ()").to_broadcast([P, NH, D]),
                        op=OP.mult,
                    )
                    nc.vector.tensor_tensor(o1[:], o1[:], o2[:], op=OP.add)
                    nc.sync.dma_start(
                        out_sl[c0 * P:(c0 + NH) * P, :].rearrange("(a p) d -> p a d", p=P),
                        o1[:],
                    )

    nc.compile()
    return nc


_NC_CACHE = {}


def _get_nc(debug=False):
    key = f"nc{debug}"
    if key not in _NC_CACHE:
        _NC_CACHE[key] = build_moe(debug)
    return _NC_CACHE[key]


def make_inputs(x, Wg, W1, b1, W2, b2):
    """Host-side sharding: per-core input maps (data-independent prep only)."""
    bf = ml_dtypes.bfloat16
    x = np.ascontiguousarray(np.asarray(x, dtype=np.float32).reshape(T, D))
    wg = np.ascontiguousarray(
        np.asarray(Wg, dtype=np.float32).reshape(DC, P, E).transpose(1, 0, 2)
    )
    p = np.arange(P)
    tm = p // E
    ee = p % E
    msame = (ee[:, None] == ee[None, :]).astype(np.float32)        # [p', p]
    mlow = (msame * (tm[:, None] < tm[None, :])).astype(np.float32)
    erow = np.tile(np.arange(E, dtype=np.float32), (P, 1))
    loall = np.tile(np.arange(P, dtype=np.float32), (P, NT, 1)).astype(bf)
    hiall = np.tile(np.arange(16, dtype=np.float32), (P, NT, 1)).astype(np.float32)
    j = np.arange(NT)
    thi = (j[None, :] * 2 + (p // 64)[:, None]).astype(np.float32)  # (j*128+p)//64
    tlo = (p % 64).astype(np.float32).reshape(P, 1)

    in_maps = []
    for e in range(E):
        w1s = np.ascontiguousarray(
            np.asarray(W1[e], dtype=np.float32)
            .reshape(DC, P, HC, P).transpose(1, 2, 0, 3).astype(bf)
        )
        w2s = np.ascontiguousarray(
            np.asarray(W2[e], dtype=np.float32).reshape(HC, P, D).transpose(1, 0, 2).astype(bf)
        )
        b1s = np.ascontiguousarray(np.asarray(b1[e], dtype=np.float32).reshape(HC, P).T)
        b2r = np.asarray(b2[e], dtype=np.float32).reshape(1, D).astype(bf)
        esel = np.zeros((P, E), dtype=np.float32)
        esel[:, e] = 1.0
        hm64 = np.zeros((P, NT), dtype=np.float32)
        hm64[:, e * NTS:(e + 1) * NTS] = 1.0
        in_maps.append({
            "xs": np.ascontiguousarray(x[e * TS:(e + 1) * TS]),
            "wg": wg, "w1s": w1s, "w2s": w2s, "b1s": b1s, "b2r": b2r,
            "esel": esel, "hm64": hm64, "erow": erow,
            "msame": msame, "mlow": mlow, "loall": loall, "hiall": hiall,
            "thi": thi, "tlo": tlo,
        })
    return in_maps


def kernel(x, Wg, W1, b1, W2, b2):
    nc = _get_nc()
    in_maps = make_inputs(x, Wg, W1, b1, W2, b2)
    res = run_bass_kernel_spmd(nc, in_maps, list(range(E)))
    out = np.concatenate([res.results[e]["out_slice"] for e in range(E)], axis=0)
    return out.reshape(B, S, D).astype(np.float32)


# revision 6
# speedup vs baseline: 1.0523x; 1.0523x over previous
"""MoE (top-2, capacity-dropped) Trainium2 kernel v2 — expert-parallel, 8 cores.

vs baseline:
- Routing logits sharded per core + AllGather (logits 32KB, xb 2.1MB).
- All-expert capacity cumsum in transposed (tm*8+e, block, token) layout;
  cross-tile offsets via two static-mask matmuls (Msame/Mlow).
- slot->token map built ON-CHIP via one-hot factored matmuls (no DRAM
  scatter/gather staging).
- FFN: weights SBUF-resident, N=512 moving operand, h staged in SBUF bf16,
  W2 per 256-slot half (PSUM: 2 h banks + 4 y banks).
- Combine: y slot-major, AllGathered in 4 chunks overlapped with FFN; home
  cores gather their tokens' <=2 expert rows and combine with weights.
"""

import numpy as np
import ml_dtypes

import concourse.bass as bass
import concourse.tile as tile
from concourse import bacc, mybir
from concourse.bass_utils import run_bass_kernel_spmd
from concourse.masks import make_identity

F32 = mybir.dt.float32
BF16 = mybir.dt.bfloat16
I16 = mybir.dt.int16
AF = mybir.ActivationFunctionType
OP = mybir.AluOpType

P = 128
E = 8
B, S, D = 2, 4096, 1024
H = 4096
T = B * S                  # 8192 tokens
C = 2048                   # capacity per expert
NT = T // P                # 64 token tiles
NTS = NT // E              # 8 token tiles per core slice
TS = T // E                # 1024 tokens per core slice
DC = D // P                # 8 d-chunks
HC = H // P                # 32 h-chunks
NB = 4                     # FFN super-blocks (512 slots each)
SB = C // NB               # 512 slots per super-block
YCH = E * SB               # rows per y AllGather chunk (4096)


def _ge_sum(nc, r2, src, levels, name):
    """acc = sum_k [src >= levels[k]] (compare cascade; all f32)."""
    acc = r2.tile(list(src.shape), F32, name=name)
    tmp = r2.tile(list(src.shape), F32, name=name + "t")
    nc.vector.tensor_scalar(acc[:], src[:], float(levels[0]), None, op0=OP.is_ge)
    for lv in levels[1:]:
        nc.vector.tensor_scalar(tmp[:], src[:], float(lv), None, op0=OP.is_ge)
        nc.vector.tensor_tensor(acc[:], acc[:], tmp[:], op=OP.add)
    return acc


def build_moe(debug=False):
    nc = bacc.Bacc("TRN2", target_bir_lowering=False, debug=False, num_devices=E)

    xs_in = nc.dram_tensor("xs", [TS, D], F32, kind="ExternalInput").ap()
    wg_in = nc.dram_tensor("wg", [P, DC, E], F32, kind="ExternalInput").ap()
    w1_in = nc.dram_tensor("w1s", [P, HC, DC, P], BF16, kind="ExternalInput").ap()
    w2_in = nc.dram_tensor("w2s", [P, HC, D], BF16, kind="ExternalInput").ap()
    b1_in = nc.dram_tensor("b1s", [P, HC], F32, kind="ExternalInput").ap()
    b2_in = nc.dram_tensor("b2r", [1, D], BF16, kind="ExternalInput").ap()
    esel_in = nc.dram_tensor("esel", [P, E], F32, kind="ExternalInput").ap()
    hm64_in = nc.dram_tensor("hm64", [P, NT], F32, kind="ExternalInput").ap()
    erow_in = nc.dram_tensor("erow", [P, E], F32, kind="ExternalInput").ap()
    msame_in = nc.dram_tensor("msame", [P, P], F32, kind="ExternalInput").ap()
    mlow_in = nc.dram_tensor("mlow", [P, P], F32, kind="ExternalInput").ap()
    lo_in = nc.dram_tensor("loall", [P, NT, P], BF16, kind="ExternalInput").ap()
    hi_in = nc.dram_tensor("hiall", [P, NT, 16], F32, kind="ExternalInput").ap()
    thi_in = nc.dram_tensor("thi", [P, NT], F32, kind="ExternalInput").ap()
    tlo_in = nc.dram_tensor("tlo", [P, 1], F32, kind="ExternalInput").ap()

    out_sl = nc.dram_tensor("out_slice", [TS, D], F32, kind="ExternalOutput").ap()

    xb_sl_dram = nc.dram_tensor("xb_slice", [TS, D], BF16)
    xb_all = nc.dram_tensor("xb_all", [T, D], BF16, addr_space="Shared")
    lg_sl_dram = nc.dram_tensor("lg_slice", [E, TS], F32)
    lg_all = nc.dram_tensor("lg_all", [E * E, TS], F32, addr_space="Shared")
    yag_in = nc.dram_tensor("yag_in", [C, D], BF16)
    yag_out = nc.dram_tensor("yag_out", [E * C, D], BF16, addr_space="Shared")

    if debug:
        dbg_lg = nc.dram_tensor("dbg_lg", [P, NT, E], F32, kind="ExternalOutput").ap()
        dbg_pos1 = nc.dram_tensor("dbg_pos1", [P, NT], F32, kind="ExternalOutput").ap()
        dbg_pos2 = nc.dram_tensor("dbg_pos2", [P, NT], F32, kind="ExternalOutput").ap()
        dbg_map = nc.dram_tensor("dbg_map", [P, 16], F32, kind="ExternalOutput").ap()
        dbg_gs1 = nc.dram_tensor("dbg_gs1", [P, NT], F32, kind="ExternalOutput").ap()
        dbg_gs2 = nc.dram_tensor("dbg_gs2", [P, NT], F32, kind="ExternalOutput").ap()
        dbg_cw1 = nc.dram_tensor("dbg_cw1", [P, NT], F32, kind="ExternalOutput").ap()
        dbg_cw2 = nc.dram_tensor("dbg_cw2", [P, NT], F32, kind="ExternalOutput").ap()

    with tile.TileContext(nc) as tc:
        with (
            tc.tile_pool(name="const", bufs=1) as const,
            tc.tile_pool(name="persist", bufs=1) as persist,
            tc.tile_pool(name="w1pool", bufs=1) as w1pool,
        ):
            # ---------------- constants ----------------
            ident = const.tile([P, P], F32)
            make_identity(nc, ident[:])
            wg_sb = const.tile([P, DC, E], F32)
            nc.sync.dma_start(wg_sb[:], wg_in[:])
            b1_sb = const.tile([P, HC], F32)
            nc.sync.dma_start(b1_sb[:], b1_in[:])
            b2_sb = const.tile([1, D], BF16)
            nc.sync.dma_start(b2_sb[:], b2_in[:])
            ones1 = const.tile([1, P], BF16)
            nc.vector.memset(ones1[:], 1.0)

            # W1 resident (hc-major layout) — scalar-queue DMA so the x-slice
            # loads on the sync queue aren't starved behind 8MB of weights
            w1_sb = w1pool.tile([P, HC, DC, P], BF16)
            for hc in range(HC):
                nc.scalar.dma_start(w1_sb[:, hc, :, :], w1_in[:, hc, :, :])

            # ---------------- P1: slice logits + bf16 cast ----------------
            with (
                tc.tile_pool(name="p1x", bufs=1) as p1x,
                tc.tile_pool(name="p1xb", bufs=3) as p1xb,
                tc.tile_pool(name="p1xt", bufs=3) as p1xt,
                tc.tile_pool(name="p1lg", bufs=2) as p1lg,
                tc.tile_pool(name="p1pst", bufs=2, space="PSUM") as p1pst,
                tc.tile_pool(name="p1psl", bufs=2, space="PSUM") as p1psl,
            ):
                # pass A: transposed logits lgT[e, tok] (critical path to AG)
                x_tiles = []
                for i in range(NTS):
                    x_sb = p1x.tile([P, D], F32, name=f"x{i}")
                    nc.sync.dma_start(x_sb[:], xs_in[i * P:(i + 1) * P, :])
                    x_tiles.append(x_sb)
                    lg_ps = p1psl.tile([E, P], F32, space="PSUM")
                    for half in range(2):
                        tr_ps = p1pst.tile([P, 4 * P], F32, space="PSUM")
                        for j in range(4):
                            dc = half * 4 + j
                            nc.tensor.matmul(
                                tr_ps[:, j * P:(j + 1) * P],
                                x_sb[:, dc * P:(dc + 1) * P],
                                ident[:],
                                is_transpose=True,
                                start=(j == 0),
                                stop=(j == 3),
                            )
                        xt_sb = p1xt.tile([P, 4 * P], F32)
                        nc.vector.tensor_copy(xt_sb[:], tr_ps[:])
                        for j in range(4):
                            dc = half * 4 + j
                            nc.tensor.matmul(
                                lg_ps[:],
                                wg_sb[:, dc, :],
                                xt_sb[:, j * P:(j + 1) * P],
                                start=(dc == 0),
                                stop=(dc == DC - 1),
                            )
                    lg_sb = p1lg.tile([E, P], F32)
                    nc.vector.tensor_copy(lg_sb[:], lg_ps[:])
                    nc.sync.dma_start(lg_sl_dram[:, i * P:(i + 1) * P], lg_sb[:])

                # highest scheduler priority: this AG gates the whole routing
                # phase and must order before the (fatter) xb AG on the CC queue
                with tc.high_priority():
                    nc.gpsimd.collective_compute(
                        "AllGather", OP.bypass, replica_groups=[list(range(E))],
                        ins=[lg_sl_dram[:].opt()], outs=[lg_all[:].opt()],
                    )

                # pass B: bf16 cast + staging (xb_all only needed at dispatch)
                for i in range(NTS):
                    xb_sb = p1xb.tile([P, D], BF16)
                    nc.vector.tensor_copy(xb_sb[:], x_tiles[i][:])
                    nc.sync.dma_start(xb_sl_dram[i * P:(i + 1) * P, :], xb_sb[:])

            nc.gpsimd.collective_compute(
                "AllGather", OP.bypass, replica_groups=[list(range(E))],
                ins=[xb_sl_dram[:].opt()], outs=[xb_all[:].opt()],
            )

            # persist tiles used across phases
            cw1_my = persist.tile([P, E], F32)
            cw2_my = persist.tile([P, E], F32)
            idx_h1 = persist.tile([P, TS // 16], I16)   # home gather idx (wrapped)
            idx_h2 = persist.tile([P, TS // 16], I16)
            idx_x = persist.tile([P, C // 16], I16)     # dispatch gather idx

            # ---------------- P2: routing (replicated, from lg_all) ------
            with (
                tc.tile_pool(name="r2", bufs=1) as r2,
                tc.tile_pool(name="ohps", bufs=2, space="PSUM") as ohps,
                tc.tile_pool(name="mmps", bufs=1, space="PSUM") as mmps,
                tc.tile_pool(name="bkps", bufs=1, space="PSUM") as bkps,
                tc.tile_pool(name="mapps", bufs=1, space="PSUM") as mapps,
            ):
                esel_sb = r2.tile([P, E], F32)
                nc.sync.dma_start(esel_sb[:], esel_in[:])
                hm64_sb = r2.tile([P, NT], F32)
                nc.sync.dma_start(hm64_sb[:], hm64_in[:])
                erow_sb = r2.tile([P, E], F32)
                nc.sync.dma_start(erow_sb[:], erow_in[:])
                msame_sb = r2.tile([P, P], F32)
                nc.sync.dma_start(msame_sb[:], msame_in[:])
                mlow_sb = r2.tile([P, P], F32)
                nc.sync.dma_start(mlow_sb[:], mlow_in[:])
                lo_sb = r2.tile([P, NT, P], BF16)
                nc.sync.dma_start(lo_sb[:], lo_in[:])
                hi_sb = r2.tile([P, NT, 16], F32)
                nc.sync.dma_start(hi_sb[:], hi_in[:])
                thi_sb = r2.tile([P, NT], F32)
                nc.sync.dma_start(thi_sb[:], thi_in[:])
                tlo_sb = r2.tile([P, 1], F32)
                nc.sync.dma_start(tlo_sb[:], tlo_in[:])

                lgx = r2.tile([E * E, TS], F32)
                nc.sync.dma_start(lgx[:], lg_all[:])
                lgt = r2.tile([P, NT, E], F32)
                lgt_v = lgt[:].rearrange("p (r j) e -> p r j e", j=NTS)
                lgtr_ps = ohps.tile([P, 4 * P], F32, space="PSUM", name="lgtr")
                for j in range(NTS):
                    nc.tensor.matmul(
                        lgtr_ps[:, j * 64:(j + 1) * 64],
                        lgx[:, j * P:(j + 1) * P],
                        ident[0:E * E, 0:E * E],
                        is_transpose=True,
                        start=(j == 0), stop=(j == NTS - 1),
                    )
                for j in range(NTS):
                    nc.vector.tensor_copy(
                        lgt_v[:, :, j, :],
                        lgtr_ps[:, j * 64:(j + 1) * 64].rearrange("p (r e) -> p r e", e=E),
                    )
                if debug:
                    nc.sync.dma_start(dbg_lg[:], lgt[:])
                # ---- top-2 (token-major) ----
                m1 = r2.tile([P, NT], F32)
                nc.vector.tensor_reduce(m1[:], lgt[:], axis=mybir.AxisListType.X, op=OP.max)
                oh1 = r2.tile([P, NT, E], F32)
                nc.vector.tensor_tensor(
                    oh1[:], lgt[:], m1[:].rearrange("p t -> p t ()").to_broadcast([P, NT, E]),
                    op=OP.is_equal,
                )
                masked = r2.tile([P, NT, E], F32)
                nc.vector.tensor_scalar(masked[:], oh1[:], -1e9, None, op0=OP.mult)
                nc.vector.tensor_tensor(masked[:], masked[:], lgt[:], op=OP.add)
                m2 = r2.tile([P, NT], F32)
                nc.vector.tensor_reduce(m2[:], masked[:], axis=mybir.AxisListType.X, op=OP.max)
                oh2 = r2.tile([P, NT, E], F32)
                nc.vector.tensor_tensor(
                    oh2[:], masked[:], m2[:].rearrange("p t -> p t ()").to_broadcast([P, NT, E]),
                    op=OP.is_equal,
                )
                delta = r2.tile([P, NT], F32)
                nc.vector.tensor_tensor(delta[:], m2[:], m1[:], op=OP.subtract)
                wr1 = r2.tile([P, NT], F32)
                nc.scalar.activation(wr1[:], delta[:], AF.Sigmoid, scale=-1.0)
                wr2 = r2.tile([P, NT], F32)
                nc.scalar.activation(wr2[:], delta[:], AF.Sigmoid)

                # ---- all-expert capacity cumsum, P2 layout (tm*8+e, b, tok) ----
                ohs = [oh1, oh2]
                csm1T = []      # token-major (cs-1)*keep*oh per rank  [P, NT, E]
                kT = []         # token-major keep*oh per rank         [P, NT, E]
                base1 = None
                for r in range(2):
                    ohT_ps = ohps.tile([P, 4 * P], F32, space="PSUM", name="ohT")
                    ohsv = ohs[r][:].rearrange("p a e -> p (a e)")
                    for b in range(4):
                        nc.tensor.matmul(
                            ohT_ps[:, b * P:(b + 1) * P],
                            ohsv[:, b * P:(b + 1) * P],
                            ident[:],
                            is_transpose=True,
                            start=(b == 0), stop=(b == 3),
                        )
                    ohT = r2.tile([P, 4, P], F32, name=f"ohTs{r}")
                    nc.vector.tensor_copy(ohT[:], ohT_ps[:].rearrange("p (b t) -> p b t", b=4))
                    ic = r2.tile([P, 4, P], F32, name=f"ic{r}")
                    for b in range(4):
                        nc.vector.tensor_tensor_scan(
                            ic[:, b, :], ohT[:, b, :], ohT[:, b, :], 0.0,
                            op0=OP.add, op1=OP.bypass,
                        )
                    cnt = r2.tile([P, 4], F32, name=f"cnt{r}")
                    nc.vector.tensor_copy(cnt[:], ic[:, :, P - 1])
                    # cross-tile offsets: same-expert block totals + intra lower
                    mm_ps = mmps.tile([P, 8], F32, space="PSUM", name="mm")
                    nc.tensor.matmul(mm_ps[:, 0:4], msame_sb[:], cnt[:], start=True, stop=False)
                    nc.tensor.matmul(mm_ps[:, 4:8], mlow_sb[:], cnt[:], start=False, stop=True)
                    mm_sb = r2.tile([P, 8], F32, name=f"mmsb{r}")
                    nc.vector.tensor_copy(mm_sb[:], mm_ps[:])
                    btot_i = r2.tile([P, 4], F32, name=f"bti{r}")
                    nc.vector.tensor_tensor_scan(
                        btot_i[:], mm_sb[:, 0:4], mm_sb[:, 0:4], 0.0,
                        op0=OP.add, op1=OP.bypass,
                    )
                    offs = r2.tile([P, 4], F32, name=f"offs{r}")
                    nc.vector.tensor_tensor(offs[:], btot_i[:], mm_sb[:, 0:4], op=OP.subtract)
                    nc.vector.tensor_tensor(offs[:], offs[:], mm_sb[:, 4:8], op=OP.add)
                    if r == 1:
                        nc.vector.tensor_scalar(offs[:], offs[:], base1[:], None, op0=OP.add)
                    cs = r2.tile([P, 4, P], F32, name=f"cs{r}")
                    for b in range(4):
                        nc.vector.tensor_scalar(
                            cs[:, b, :], ic[:, b, :], offs[:, b:b + 1], None, op0=OP.add
                        )
                    if r == 0:
                        # rank-1 base: min(total rank-0 assigned per expert, C);
                        # mm_sb[:, 0:4] holds per-expert block totals (Msame).
                        n0 = r2.tile([P, 1], F32)
                        nc.vector.tensor_reduce(n0[:], mm_sb[:, 0:4], axis=mybir.AxisListType.X, op=OP.add)
                        base1 = r2.tile([P, 1], F32)
                        nc.vector.tensor_scalar(base1[:], n0[:], float(C), None, op0=OP.min)
                    keep = r2.tile([P, 4, P], F32, name=f"keep{r}")
                    nc.vector.tensor_scalar(keep[:], cs[:], float(C), None, op0=OP.is_le)
                    kk = r2.tile([P, 4, P], F32, name=f"kk{r}")
                    nc.vector.tensor_tensor(kk[:], keep[:], ohT[:], op=OP.mult)
                    ksl = r2.tile([P, 4, P], F32, name=f"ksl{r}")
                    nc.vector.tensor_scalar(ksl[:], cs[:], -1.0, None, op0=OP.add)
                    nc.vector.tensor_tensor(ksl[:], ksl[:], kk[:], op=OP.mult)
                    # transpose back to token-major (two 1-bank psum tiles)
                    bk1 = bkps.tile([P, 4 * P], F32, space="PSUM", name="bk1")
                    bk2 = bkps.tile([P, 4 * P], F32, space="PSUM", name="bk2")
                    for b in range(4):
                        nc.tensor.matmul(
                            bk1[:, b * P:(b + 1) * P], ksl[:, b, :], ident[:],
                            is_transpose=True, start=(b == 0), stop=(b == 3),
                        )
                    for b in range(4):
                        nc.tensor.matmul(
                            bk2[:, b * P:(b + 1) * P], kk[:, b, :], ident[:],
                            is_transpose=True, start=(b == 0), stop=(b == 3),
                        )
                    cT = r2.tile([P, NT, E], F32, name=f"cT{r}")
                    nc.vector.tensor_copy(cT[:], bk1[:].rearrange("p (a e) -> p a e", e=E))
                    kTr = r2.tile([P, NT, E], F32, name=f"kTr{r}")
                    nc.vector.tensor_copy(kTr[:], bk2[:].rearrange("p (a e) -> p a e", e=E))
                    csm1T.append(cT)
                    kT.append(kTr)

                # ---- home-side indices (token-major, all tiles) ----
                esel_b = esel_sb[:].rearrange("p e -> p () e").to_broadcast([P, NT, E])
                erow_b = erow_sb[:].rearrange("p e -> p () e").to_broadcast([P, NT, E])
                tmp3 = r2.tile([P, NT, E], F32)
                gs = []
                cwf = []
                for r in range(2):
                    pos = r2.tile([P, NT], F32, name=f"pos{r}")
                    nc.vector.tensor_reduce(pos[:], csm1T[r][:], axis=mybir.AxisListType.X, op=OP.add)
                    keep_s = r2.tile([P, NT], F32, name=f"ks{r}")
                    nc.vector.tensor_reduce(keep_s[:], kT[r][:], axis=mybir.AxisListType.X, op=OP.max)
                    nc.vector.tensor_tensor(tmp3[:], ohs[r][:], erow_b, op=OP.mult)
                    es = r2.tile([P, NT], F32, name=f"es{r}")
                    nc.vector.tensor_reduce(es[:], tmp3[:], axis=mybir.AxisListType.X, op=OP.max)
                    # AG row index: 2048*(s//256) + 256*e + s%256
                    q = _ge_sum(nc, r2, pos, [256.0 * k for k in range(1, 8)], f"q{r}")
                    g = r2.tile([P, NT], F32, name=f"g{r}")
                    rem = r2.tile([P, NT], F32, name=f"rm{r}")
                    nc.vector.tensor_scalar(rem[:], q[:], -256.0, None, op0=OP.mult)
                    nc.vector.tensor_tensor(rem[:], rem[:], pos[:], op=OP.add)
                    nc.vector.tensor_scalar(g[:], q[:], 2048.0, None, op0=OP.mult)
                    t2 = r2.tile([P, NT], F32, name=f"t2{r}")
                    nc.vector.tensor_scalar(t2[:], es[:], 256.0, None, op0=OP.mult)
                    nc.vector.tensor_tensor(g[:], g[:], t2[:], op=OP.add)
                    nc.vector.tensor_tensor(g[:], g[:], rem[:], op=OP.add)
                    cw = r2.tile([P, NT], F32, name=f"cw{r}")
                    wsrc = wr1 if r == 0 else wr2
                    nc.vector.tensor_tensor(cw[:], wsrc[:], keep_s[:], op=OP.mult)
                    gs.append(g)
                    cwf.append(cw)
                    if debug:
                        nc.sync.dma_start([dbg_gs1, dbg_gs2][r][:], g[:])
                        nc.sync.dma_start([dbg_cw1, dbg_cw2][r][:], cw[:])
                        nc.sync.dma_start([dbg_pos1, dbg_pos2][r][:], pos[:])

                # select MY home block (hm64 mask + log-fold), build wrapped idx
                for r in range(2):
                    msk_g = r2.tile([P, NT], F32, name=f"mg{r}")
                    nc.vector.tensor_tensor(msk_g[:], gs[r][:], hm64_sb[:], op=OP.mult)
                    msk_c = r2.tile([P, NT], F32, name=f"mc{r}")
                    nc.vector.tensor_tensor(msk_c[:], cwf[r][:], hm64_sb[:], op=OP.mult)
                    for half in (32, 16, 8):
                        nc.vector.tensor_tensor(
                            msk_g[:, 0:half], msk_g[:, 0:half], msk_g[:, half:2 * half], op=OP.add
                        )
                        nc.vector.tensor_tensor(
                            msk_c[:, 0:half], msk_c[:, 0:half], msk_c[:, half:2 * half], op=OP.add
                        )
                    nc.vector.tensor_copy([cw1_my, cw2_my][r][:], msk_c[:, 0:E])
                    gi = r2.tile([P, E], I16, name=f"gi{r}")
                    nc.vector.tensor_copy(gi[:], msk_g[:, 0:E])
                    sh16 = [(i + 16) % 32 for i in range(32)]
                    gish = r2.tile([P, E], I16, name=f"gish{r}")
                    nc.vector.stream_shuffle(gish[:], gi[:], sh16)
                    idxh = [idx_h1, idx_h2][r]
                    idxh_v = idxh[0:16, :].rearrange("r (c q) -> r c q", q=8)
                    for qq in range(8):
                        src = gi if qq % 2 == 0 else gish
                        nc.vector.tensor_copy(
                            idxh_v[:, :, qq],
                            src[(qq // 2) * 32:(qq // 2) * 32 + 16, :],
                        )
                    for k in range(1, 8):
                        nc.sync.dma_start(idxh[16 * k:16 * (k + 1), :], idxh[0:16, :])

                # ---- expert-side slot->token map (my expert) ----
                ksl_e = r2.tile([P, NT], F32)
                k_e = r2.tile([P, NT], F32)
                acc = r2.tile([P, NT], F32)
                for r in range(2):
                    nc.vector.tensor_tensor(tmp3[:], csm1T[r][:], esel_b, op=OP.mult)
                    nc.vector.tensor_reduce(
                        (acc if r else ksl_e)[:], tmp3[:], axis=mybir.AxisListType.X, op=OP.add
                    )
                    if r:
                        nc.vector.tensor_tensor(ksl_e[:], ksl_e[:], acc[:], op=OP.add)
                    nc.vector.tensor_tensor(tmp3[:], kT[r][:], esel_b, op=OP.mult)
                    nc.vector.tensor_reduce(
                        (acc if r else k_e)[:], tmp3[:], axis=mybir.AxisListType.X, op=OP.max
                    )
                    if r:
                        nc.vector.tensor_tensor(k_e[:], k_e[:], acc[:], op=OP.max)
                # chi = s//128 in [0,16), remc = s%128 via two-level cascade
                q8 = _ge_sum(nc, r2, ksl_e, [512.0, 1024.0, 1536.0], "q8")
                s1 = r2.tile([P, NT], F32)
                nc.vector.tensor_scalar(s1[:], q8[:], -512.0, None, op0=OP.mult)
                nc.vector.tensor_tensor(s1[:], s1[:], ksl_e[:], op=OP.add)
                c3 = _ge_sum(nc, r2, s1, [128.0, 256.0, 384.0], "c3")
                chi = r2.tile([P, NT], F32)
                nc.vector.tensor_scalar(chi[:], q8[:], 4.0, None, op0=OP.mult)
                nc.vector.tensor_tensor(chi[:], chi[:], c3[:], op=OP.add)
                remc = r2.tile([P, NT], F32)
                nc.vector.tensor_scalar(remc[:], c3[:], -128.0, None, op0=OP.mult)
                nc.vector.tensor_tensor(remc[:], remc[:], s1[:], op=OP.add)
                ktlo = r2.tile([P, NT], F32)
                nc.vector.tensor_scalar(ktlo[:], k_e[:], tlo_sb[:], None, op0=OP.mult)
                kthi = r2.tile([P, NT], F32)
                nc.vector.tensor_tensor(kthi[:], k_e[:], thi_sb[:], op=OP.mult)

                o_all = r2.tile([P, NT, P], BF16)   # [s%128 == lo]
                nc.vector.tensor_tensor(
                    o_all[:], lo_sb[:],
                    remc[:].rearrange("p t -> p t ()").to_broadcast([P, NT, P]),
                    op=OP.is_equal,
                )
                v0 = r2.tile([P, NT, 16], F32)
                nc.vector.tensor_tensor(
                    v0[:], hi_sb[:],
                    chi[:].rearrange("p t -> p t ()").to_broadcast([P, NT, 16]),
                    op=OP.is_equal,
                )
                v_all = r2.tile([P, NT, 2, 16], BF16)
                nc.vector.tensor_tensor(
                    v_all[:, :, 0, :], v0[:],
                    ktlo[:].rearrange("p t -> p t ()").to_broadcast([P, NT, 16]),
                    op=OP.mult,
                )
                nc.vector.tensor_tensor(
                    v_all[:, :, 1, :], v0[:],
                    kthi[:].rearrange("p t -> p t ()").to_broadcast([P, NT, 16]),
                    op=OP.mult,
                )
                map_ps = mapps.tile([P, 2, 16], F32, space="PSUM")
                for j in range(NT):
                    nc.tensor.matmul(
                        map_ps[:].rearrange("p a b -> p (a b)"),
                        o_all[:, j, :],
                        v_all[:, j, :, :].rearrange("p a b -> p (a b)"),
                        start=(j == 0), stop=(j == NT - 1),
                    )
                map_sb = r2.tile([P, 2, 16], F32)
                nc.vector.tensor_copy(map_sb[:], map_ps[:])
                map_tok = r2.tile([P, 16], F32)
                nc.vector.tensor_scalar(map_tok[:], map_sb[:, 1, :], 64.0, None, op0=OP.mult)
                nc.vector.tensor_tensor(map_tok[:], map_tok[:], map_sb[:, 0, :], op=OP.add)
                if debug:
                    nc.sync.dma_start(dbg_map[:], map_tok[:])
                mi = r2.tile([P, 16], I16)
                nc.vector.tensor_copy(mi[:], map_tok[:])
                sh16 = [(i + 16) % 32 for i in range(32)]
                mish = r2.tile([P, 16], I16)
                nc.vector.stream_shuffle(mish[:], mi[:], sh16)
                idxx_v = idx_x[0:16, :].rearrange("r (h q) -> r h q", q=8)
                for qq in range(8):
                    src = mi if qq % 2 == 0 else mish
                    nc.vector.tensor_copy(
                        idxx_v[:, :, qq],
                        src[(qq // 2) * 32:(qq // 2) * 32 + 16, :],
                    )
                for k in range(1, 8):
                    nc.sync.dma_start(idx_x[16 * k:16 * (k + 1), :], idx_x[0:16, :])

            # ---------------- P3: FFN ----------------
            with (
                tc.tile_pool(name="w2pool", bufs=1) as w2pool,
                tc.tile_pool(name="xte", bufs=1) as xtep,
                tc.tile_pool(name="hall", bufs=1) as hallp,
                tc.tile_pool(name="ypool", bufs=1) as ypool,
                tc.tile_pool(name="hps", bufs=2, space="PSUM") as hps,
                tc.tile_pool(name="yps", bufs=1, space="PSUM") as yps,
            ):
                w2_sb = w2pool.tile([P, HC, D], BF16)
                for hc in range(HC):
                    nc.sync.dma_start(w2_sb[:, hc, :], w2_in[:, hc, :])
                h_all = hallp.tile([P, HC, SB], BF16)
                # all dispatch gathers up-front: gpsimd queue is in-order and
                # collective triggers block it, so gathers must precede them
                xTes = []
                for sb in range(NB):
                    xTe = xtep.tile([P, DC, SB], BF16, name=f"xTe{sb}")
                    nc.gpsimd.dma_gather(
                        out_ap=xTe[:],
                        in_ap=xb_all[:],
                        idxs_ap=idx_x[:, sb * (SB // 16):(sb + 1) * (SB // 16)],
                        num_idxs=SB, num_idxs_reg=SB, elem_size=D, transpose=True,
                    )
                    xTes.append(xTe)
                for sb in range(NB):
                    xTe = xTes[sb]
                    for hc in range(HC):
                        h_ps = hps.tile([P, SB], F32, space="PSUM", name="hps")
                        for dc in range(DC):
                            nc.tensor.matmul(
                                h_ps[:],
                                w1_sb[:, hc, dc, :],
                                xTe[:, dc, :],
                                start=(dc == 0), stop=(dc == DC - 1),
                            )
                        nc.scalar.activation(
                            h_all[:, hc, :], h_ps[:], AF.Gelu_apprx_tanh,
                            bias=b1_sb[:, hc:hc + 1],
                        )
                    for half in range(2):
                        y_ts = [
                            [yps.tile([P, 512], F32, space="PSUM", name=f"y{st}{dg}") for dg in range(2)]
                            for st in range(2)
                        ]
                        for hc in range(HC):
                            for st in range(2):
                                so = half * 256 + st * P
                                for dg in range(2):
                                    nc.tensor.matmul(
                                        y_ts[st][dg][:],
                                        h_all[:, hc, so:so + P],
                                        w2_sb[:, hc, dg * 512:(dg + 1) * 512],
                                        start=(hc == 0), stop=False,
                                    )
                        y_sb = ypool.tile([P, 2, D], BF16, name="ysb")
                        for st in range(2):
                            for dg in range(2):
                                nc.tensor.matmul(
                                    y_ts[st][dg][:], ones1[:],
                                    b2_sb[:, dg * 512:(dg + 1) * 512],
                                    start=False, stop=True,
                                )
                                nc.scalar.activation(
                                    y_sb[:, st, dg * 512:(dg + 1) * 512],
                                    y_ts[st][dg][:], AF.Copy,
                                )
                        r0 = sb * SB + half * 256
                        nc.sync.dma_start(
                            yag_in[r0:r0 + 256, :].rearrange("(s p) d -> p s d", p=P),
                            y_sb[:],
                        )
                        ch = 2 * sb + half
                        nc.gpsimd.collective_compute(
                            "AllGather", OP.bypass, replica_groups=[list(range(E))],
                            ins=[yag_in[ch * 256:(ch + 1) * 256, :].opt()],
                            outs=[yag_out[ch * 2048:(ch + 1) * 2048, :].opt()],
                        )

            # ---------------- P4: home combine (2 pipelined halves) -------
            NH = NTS // 2
            with tc.tile_pool(name="homep", bufs=2) as homep:
                for hh in range(2):
                    c0 = hh * NH
                    g1 = homep.tile([P, NH, D], BF16, name="g1")
                    nc.gpsimd.dma_gather(
                        out_ap=g1[:], in_ap=yag_out[:],
                        idxs_ap=idx_h1[:, c0 * 8:(c0 + NH) * 8],
                        num_idxs=NH * P, num_idxs_reg=NH * P, elem_size=D,
                    )
                    g2 = homep.tile([P, NH, D], BF16, name="g2")
                    nc.gpsimd.dma_gather(
                        out_ap=g2[:], in_ap=yag_out[:],
                        idxs_ap=idx_h2[:, c0 * 8:(c0 + NH) * 8],
                        num_idxs=NH * P, num_idxs_reg=NH * P, elem_size=D,
                    )
                    o1 = homep.tile([P, NH, D], F32, name="o1")
                    nc.vector.tensor_tensor(
                        o1[:], g1[:],
                        cw1_my[:, c0:c0 + NH].rearrange("p c -> p c ()").to_broadcast([P, NH, D]),
                        op=OP.mult,
                    )
                    o2 = homep.tile([P, NH, D], F32, name="o2")
                    nc.vector.tensor_tensor(
                        o2[:], g2[:],
                        cw2_my[:, c0:c0 + NH].rearrange("p c -> p c ()").to_broadcast([P, NH, D]),
                        op=OP.mult,
                    )
                    nc.vector.tensor_tensor(o1[:], o1[:], o2[:], op=OP.add)
                    nc.sync.dma_start(
                        out_sl[c0 * P:(c0 + NH) * P, :].rearrange("(a p) d -> p a d", p=P),
                        o1[:],
                    )

    nc.compile()
    return nc


_NC_CACHE = {}


def _get_nc(debug=False):
    key = f"nc{debug}"
    if key not in _NC_CACHE:
        _NC_CACHE[key] = build_moe(debug)
    return _NC_CACHE[key]


def make_inputs(x, Wg, W1, b1, W2, b2):
    """Host-side sharding: per-core input maps (data-independent prep only)."""
    bf = ml_dtypes.bfloat16
    x = np.ascontiguousarray(np.asarray(x, dtype=np.float32).reshape(T, D))
    wg = np.ascontiguousarray(
        np.asarray(Wg, dtype=np.float32).reshape(DC, P, E).transpose(1, 0, 2)
    )
    p = np.arange(P)
    tm = p // E
    ee = p % E
    msame = (ee[:, None] == ee[None, :]).astype(np.float32)        # [p', p]
    mlow = (msame * (tm[:, None] < tm[None, :])).astype(np.float32)
    erow = np.tile(np.arange(E, dtype=np.float32), (P, 1))
    loall = np.tile(np.arange(P, dtype=np.float32), (P, NT, 1)).astype(bf)
    hiall = np.tile(np.arange(16, dtype=np.float32), (P, NT, 1)).astype(np.float32)
    j = np.arange(NT)
    thi = (j[None, :] * 2 + (p // 64)[:, None]).astype(np.float32)  # (j*128+p)//64
    tlo = (p % 64).astype(np.float32).reshape(P, 1)

    in_maps = []
    for e in range(E):
        w1s = np.ascontiguousarray(
            np.asarray(W1[e], dtype=np.float32)
            .reshape(DC, P, HC, P).transpose(1, 2, 0, 3).astype(bf)
        )
        w2s = np.ascontiguousarray(
            np.asarray(W2[e], dtype=np.float32).reshape(HC, P, D).transpose(1, 0, 2).astype(bf)
        )
        b1s = np.ascontiguousarray(np.asarray(b1[e], dtype=np.float32).reshape(HC, P).T)
        b2r = np.asarray(b2[e], dtype=np.float32).reshape(1, D).astype(bf)
        esel = np.zeros((P, E), dtype=np.float32)
        esel[:, e] = 1.0
        hm64 = np.zeros((P, NT), dtype=np.float32)
        hm64[:, e * NTS:(e + 1) * NTS] = 1.0
        in_maps.append({
            "xs": np.ascontiguousarray(x[e * TS:(e + 1) * TS]),
            "wg": wg, "w1s": w1s, "w2s": w2s, "b1s": b1s, "b2r": b2r,
            "esel": esel, "hm64": hm64, "erow": erow,
            "msame": msame, "mlow": mlow, "loall": loall, "hiall": hiall,
            "thi": thi, "tlo": tlo,
        })
    return in_maps


def kernel(x, Wg, W1, b1, W2, b2):
    nc = _get_nc()
    in_maps = make_inputs(x, Wg, W1, b1, W2, b2)
    res = run_bass_kernel_spmd(nc, in_maps, list(range(E)))
    out = np.concatenate([res.results[e]["out_slice"] for e in range(E)], axis=0)
    return out.reshape(B, S, D).astype(np.float32)


# revision 7
# speedup vs baseline: 1.1167x; 1.0612x over previous
"""MoE (top-2, capacity-dropped) Trainium2 kernel v2 — expert-parallel, 8 cores.

vs baseline:
- Routing logits sharded per core + AllGather (logits 32KB, xb 2.1MB).
- All-expert capacity cumsum in transposed (tm*8+e, block, token) layout;
  cross-tile offsets via two static-mask matmuls (Msame/Mlow).
- slot->token map built ON-CHIP via one-hot factored matmuls (no DRAM
  scatter/gather staging).
- FFN: weights SBUF-resident, N=512 moving operand, h staged in SBUF bf16,
  W2 per 256-slot half (PSUM: 2 h banks + 4 y banks).
- Combine: y slot-major, AllGathered in 4 chunks overlapped with FFN; home
  cores gather their tokens' <=2 expert rows and combine with weights.
"""

import numpy as np
import ml_dtypes

import concourse.bass as bass
import concourse.tile as tile
from concourse import bacc, mybir
from concourse.bass_utils import run_bass_kernel_spmd
from concourse.masks import make_identity

F32 = mybir.dt.float32
BF16 = mybir.dt.bfloat16
I16 = mybir.dt.int16
AF = mybir.ActivationFunctionType
OP = mybir.AluOpType

P = 128
E = 8
B, S, D = 2, 4096, 1024
H = 4096
T = B * S                  # 8192 tokens
C = 2048                   # capacity per expert
NT = T // P                # 64 token tiles
NTS = NT // E              # 8 token tiles per core slice
TS = T // E                # 1024 tokens per core slice
DC = D // P                # 8 d-chunks
HC = H // P                # 32 h-chunks
NB = 4                     # FFN super-blocks (512 slots each)
SB = C // NB               # 512 slots per super-block
YCH = E * SB               # rows per y AllGather chunk (4096)


def _ge_sum(nc, r2, src, levels, name):
    """acc = sum_k [src >= levels[k]] (compare cascade; all f32)."""
    acc = r2.tile(list(src.shape), F32, name=name)
    tmp = r2.tile(list(src.shape), F32, name=name + "t")
    nc.vector.tensor_scalar(acc[:], src[:], float(levels[0]), None, op0=OP.is_ge)
    for lv in levels[1:]:
        nc.vector.tensor_scalar(tmp[:], src[:], float(lv), None, op0=OP.is_ge)
        nc.vector.tensor_tensor(acc[:], acc[:], tmp[:], op=OP.add)
    return acc


def build_moe(debug=False):
    nc = bacc.Bacc("TRN2", target_bir_lowering=False, debug=False, num_devices=E)

    xs_in = nc.dram_tensor("xs", [TS, D], F32, kind="ExternalInput").ap()
    wg_in = nc.dram_tensor("wg", [P, DC, E], F32, kind="ExternalInput").ap()
    w1_in = nc.dram_tensor("w1s", [P, HC, DC, P], BF16, kind="ExternalInput").ap()
    w2_in = nc.dram_tensor("w2s", [P, HC, D], BF16, kind="ExternalInput").ap()
    b1_in = nc.dram_tensor("b1s", [P, HC], F32, kind="ExternalInput").ap()
    b2_in = nc.dram_tensor("b2r", [1, D], BF16, kind="ExternalInput").ap()
    esel_in = nc.dram_tensor("esel", [P, E], F32, kind="ExternalInput").ap()
    hm64_in = nc.dram_tensor("hm64", [P, NT], F32, kind="ExternalInput").ap()
    erow_in = nc.dram_tensor("erow", [P, E], F32, kind="ExternalInput").ap()
    msame_in = nc.dram_tensor("msame", [P, P], F32, kind="ExternalInput").ap()
    mlow_in = nc.dram_tensor("mlow", [P, P], F32, kind="ExternalInput").ap()
    lo_in = nc.dram_tensor("loall", [P, NT, P], BF16, kind="ExternalInput").ap()
    hi_in = nc.dram_tensor("hiall", [P, NT, 16], F32, kind="ExternalInput").ap()
    thi_in = nc.dram_tensor("thi", [P, NT], F32, kind="ExternalInput").ap()
    tlo_in = nc.dram_tensor("tlo", [P, 1], F32, kind="ExternalInput").ap()

    out_sl = nc.dram_tensor("out_slice", [TS, D], F32, kind="ExternalOutput").ap()

    xb_sl_dram = nc.dram_tensor("xb_slice", [TS, D], BF16)
    xb_all = nc.dram_tensor("xb_all", [T, D], BF16, addr_space="Shared")
    lg_sl_dram = nc.dram_tensor("lg_slice", [E, TS], F32)
    lg_all = nc.dram_tensor("lg_all", [E * E, TS], F32, addr_space="Shared")
    yag_in = nc.dram_tensor("yag_in", [C, D], BF16)
    yag_out = nc.dram_tensor("yag_out", [E * C, D], BF16, addr_space="Shared")

    if debug:
        dbg_lg = nc.dram_tensor("dbg_lg", [P, NT, E], F32, kind="ExternalOutput").ap()
        dbg_pos1 = nc.dram_tensor("dbg_pos1", [P, NT], F32, kind="ExternalOutput").ap()
        dbg_pos2 = nc.dram_tensor("dbg_pos2", [P, NT], F32, kind="ExternalOutput").ap()
        dbg_map = nc.dram_tensor("dbg_map", [P, 16], F32, kind="ExternalOutput").ap()
        dbg_gs1 = nc.dram_tensor("dbg_gs1", [P, NT], F32, kind="ExternalOutput").ap()
        dbg_gs2 = nc.dram_tensor("dbg_gs2", [P, NT], F32, kind="ExternalOutput").ap()
        dbg_cw1 = nc.dram_tensor("dbg_cw1", [P, NT], F32, kind="ExternalOutput").ap()
        dbg_cw2 = nc.dram_tensor("dbg_cw2", [P, NT], F32, kind="ExternalOutput").ap()

    with tile.TileContext(nc) as tc:
        with (
            tc.tile_pool(name="const", bufs=1) as const,
            tc.tile_pool(name="persist", bufs=1) as persist,
            tc.tile_pool(name="w1pool", bufs=1) as w1pool,
        ):
            # ---------------- constants ----------------
            ident = const.tile([P, P], F32)
            make_identity(nc, ident[:])
            wg_sb = const.tile([P, DC, E], F32)
            nc.sync.dma_start(wg_sb[:], wg_in[:])
            b1_sb = const.tile([P, HC], F32)
            nc.sync.dma_start(b1_sb[:], b1_in[:])
            b2_sb = const.tile([1, D], BF16)
            nc.sync.dma_start(b2_sb[:], b2_in[:])
            ones1 = const.tile([1, P], BF16)
            nc.vector.memset(ones1[:], 1.0)

            # W1 resident (hc-major layout) — scalar-queue DMA so the x-slice
            # loads on the sync queue aren't starved behind 8MB of weights
            w1_sb = w1pool.tile([P, HC, DC, P], BF16)
            for hc in range(HC):
                nc.scalar.dma_start(w1_sb[:, hc, :, :], w1_in[:, hc, :, :])

            # ---------------- P1: slice logits + bf16 cast ----------------
            with (
                tc.tile_pool(name="p1x", bufs=1) as p1x,
                tc.tile_pool(name="p1xb", bufs=3) as p1xb,
                tc.tile_pool(name="p1xt", bufs=3) as p1xt,
                tc.tile_pool(name="p1lg", bufs=2) as p1lg,
                tc.tile_pool(name="p1pst", bufs=2, space="PSUM") as p1pst,
                tc.tile_pool(name="p1psl", bufs=2, space="PSUM") as p1psl,
            ):
                # pass A: transposed logits lgT[e, tok] (critical path to AG)
                x_tiles = []
                for i in range(NTS):
                    x_sb = p1x.tile([P, D], F32, name=f"x{i}")
                    nc.sync.dma_start(x_sb[:], xs_in[i * P:(i + 1) * P, :])
                    x_tiles.append(x_sb)
                    lg_ps = p1psl.tile([E, P], F32, space="PSUM")
                    for half in range(2):
                        tr_ps = p1pst.tile([P, 4 * P], F32, space="PSUM")
                        for j in range(4):
                            dc = half * 4 + j
                            nc.tensor.matmul(
                                tr_ps[:, j * P:(j + 1) * P],
                                x_sb[:, dc * P:(dc + 1) * P],
                                ident[:],
                                is_transpose=True,
                                start=(j == 0),
                                stop=(j == 3),
                            )
                        xt_sb = p1xt.tile([P, 4 * P], F32)
                        nc.vector.tensor_copy(xt_sb[:], tr_ps[:])
                        for j in range(4):
                            dc = half * 4 + j
                            nc.tensor.matmul(
                                lg_ps[:],
                                wg_sb[:, dc, :],
                                xt_sb[:, j * P:(j + 1) * P],
                                start=(dc == 0),
                                stop=(dc == DC - 1),
                            )
                    lg_sb = p1lg.tile([E, P], F32)
                    nc.vector.tensor_copy(lg_sb[:], lg_ps[:])
                    nc.sync.dma_start(lg_sl_dram[:, i * P:(i + 1) * P], lg_sb[:])

                # this AG gates the whole routing phase: the scheduler fence
                # below keeps its trigger ahead of the (fatter) xb AG on the
                # in-order CC queue
                nc.gpsimd.collective_compute(
                    "AllGather", OP.bypass, replica_groups=[list(range(E))],
                    ins=[lg_sl_dram[:].opt()], outs=[lg_all[:].opt()],
                )
                tc.no_sync_barrier()

                # pass B: bf16 cast + staging (xb_all only needed at dispatch)
                for i in range(NTS):
                    xb_sb = p1xb.tile([P, D], BF16)
                    nc.vector.tensor_copy(xb_sb[:], x_tiles[i][:])
                    nc.sync.dma_start(xb_sl_dram[i * P:(i + 1) * P, :], xb_sb[:])

            nc.gpsimd.collective_compute(
                "AllGather", OP.bypass, replica_groups=[list(range(E))],
                ins=[xb_sl_dram[:].opt()], outs=[xb_all[:].opt()],
            )

            # persist tiles used across phases
            cw1_my = persist.tile([P, E], F32)
            cw2_my = persist.tile([P, E], F32)
            idx_h1 = persist.tile([P, TS // 16], I16)   # home gather idx (wrapped)
            idx_h2 = persist.tile([P, TS // 16], I16)
            idx_x = persist.tile([P, C // 16], I16)     # dispatch gather idx

            # ---------------- P2: routing (replicated, from lg_all) ------
            with (
                tc.tile_pool(name="r2", bufs=1) as r2,
                tc.tile_pool(name="ohps", bufs=2, space="PSUM") as ohps,
                tc.tile_pool(name="mmps", bufs=1, space="PSUM") as mmps,
                tc.tile_pool(name="bkps", bufs=1, space="PSUM") as bkps,
                tc.tile_pool(name="mapps", bufs=1, space="PSUM") as mapps,
            ):
                esel_sb = r2.tile([P, E], F32)
                nc.sync.dma_start(esel_sb[:], esel_in[:])
                hm64_sb = r2.tile([P, NT], F32)
                nc.sync.dma_start(hm64_sb[:], hm64_in[:])
                erow_sb = r2.tile([P, E], F32)
                nc.sync.dma_start(erow_sb[:], erow_in[:])
                msame_sb = r2.tile([P, P], F32)
                nc.sync.dma_start(msame_sb[:], msame_in[:])
                mlow_sb = r2.tile([P, P], F32)
                nc.sync.dma_start(mlow_sb[:], mlow_in[:])
                lo_sb = r2.tile([P, NT, P], BF16)
                nc.sync.dma_start(lo_sb[:], lo_in[:])
                hi_sb = r2.tile([P, NT, 16], F32)
                nc.sync.dma_start(hi_sb[:], hi_in[:])
                thi_sb = r2.tile([P, NT], F32)
                nc.sync.dma_start(thi_sb[:], thi_in[:])
                tlo_sb = r2.tile([P, 1], F32)
                nc.sync.dma_start(tlo_sb[:], tlo_in[:])

                lgx = r2.tile([E * E, TS], F32)
                nc.sync.dma_start(lgx[:], lg_all[:])
                lgt = r2.tile([P, NT, E], F32)
                lgt_v = lgt[:].rearrange("p (r j) e -> p r j e", j=NTS)
                lgtr_ps = ohps.tile([P, 4 * P], F32, space="PSUM", name="lgtr")
                for j in range(NTS):
                    nc.tensor.matmul(
                        lgtr_ps[:, j * 64:(j + 1) * 64],
                        lgx[:, j * P:(j + 1) * P],
                        ident[0:E * E, 0:E * E],
                        is_transpose=True,
                        start=(j == 0), stop=(j == NTS - 1),
                    )
                for j in range(NTS):
                    nc.vector.tensor_copy(
                        lgt_v[:, :, j, :],
                        lgtr_ps[:, j * 64:(j + 1) * 64].rearrange("p (r e) -> p r e", e=E),
                    )
                if debug:
                    nc.sync.dma_start(dbg_lg[:], lgt[:])
                # ---- top-2 (token-major) ----
                m1 = r2.tile([P, NT], F32)
                nc.vector.tensor_reduce(m1[:], lgt[:], axis=mybir.AxisListType.X, op=OP.max)
                oh1 = r2.tile([P, NT, E], F32)
                nc.vector.tensor_tensor(
                    oh1[:], lgt[:], m1[:].rearrange("p t -> p t ()").to_broadcast([P, NT, E]),
                    op=OP.is_equal,
                )
                masked = r2.tile([P, NT, E], F32)
                nc.vector.tensor_scalar(masked[:], oh1[:], -1e9, None, op0=OP.mult)
                nc.vector.tensor_tensor(masked[:], masked[:], lgt[:], op=OP.add)
                m2 = r2.tile([P, NT], F32)
                nc.vector.tensor_reduce(m2[:], masked[:], axis=mybir.AxisListType.X, op=OP.max)
                oh2 = r2.tile([P, NT, E], F32)
                nc.vector.tensor_tensor(
                    oh2[:], masked[:], m2[:].rearrange("p t -> p t ()").to_broadcast([P, NT, E]),
                    op=OP.is_equal,
                )
                delta = r2.tile([P, NT], F32)
                nc.vector.tensor_tensor(delta[:], m2[:], m1[:], op=OP.subtract)
                wr1 = r2.tile([P, NT], F32)
                nc.scalar.activation(wr1[:], delta[:], AF.Sigmoid, scale=-1.0)
                wr2 = r2.tile([P, NT], F32)
                nc.scalar.activation(wr2[:], delta[:], AF.Sigmoid)

                # ---- all-expert capacity cumsum, P2 layout (tm*8+e, b, tok) ----
                ohs = [oh1, oh2]
                csm1T = []      # token-major (cs-1)*keep*oh per rank  [P, NT, E]
                kT = []         # token-major keep*oh per rank         [P, NT, E]
                base1 = None
                for r in range(2):
                    ohT_ps = ohps.tile([P, 4 * P], F32, space="PSUM", name="ohT")
                    ohsv = ohs[r][:].rearrange("p a e -> p (a e)")
                    for b in range(4):
                        nc.tensor.matmul(
                            ohT_ps[:, b * P:(b + 1) * P],
                            ohsv[:, b * P:(b + 1) * P],
                            ident[:],
                            is_transpose=True,
                            start=(b == 0), stop=(b == 3),
                        )
                    ohT = r2.tile([P, 4, P], F32, name=f"ohTs{r}")
                    nc.vector.tensor_copy(ohT[:], ohT_ps[:].rearrange("p (b t) -> p b t", b=4))
                    ic = r2.tile([P, 4, P], F32, name=f"ic{r}")
                    for b in range(4):
                        nc.vector.tensor_tensor_scan(
                            ic[:, b, :], ohT[:, b, :], ohT[:, b, :], 0.0,
                            op0=OP.add, op1=OP.bypass,
                        )
                    cnt = r2.tile([P, 4], F32, name=f"cnt{r}")
                    nc.vector.tensor_copy(cnt[:], ic[:, :, P - 1])
                    # cross-tile offsets: same-expert block totals + intra lower
                    mm_ps = mmps.tile([P, 8], F32, space="PSUM", name="mm")
                    nc.tensor.matmul(mm_ps[:, 0:4], msame_sb[:], cnt[:], start=True, stop=False)
                    nc.tensor.matmul(mm_ps[:, 4:8], mlow_sb[:], cnt[:], start=False, stop=True)
                    mm_sb = r2.tile([P, 8], F32, name=f"mmsb{r}")
                    nc.vector.tensor_copy(mm_sb[:], mm_ps[:])
                    btot_i = r2.tile([P, 4], F32, name=f"bti{r}")
                    nc.vector.tensor_tensor_scan(
                        btot_i[:], mm_sb[:, 0:4], mm_sb[:, 0:4], 0.0,
                        op0=OP.add, op1=OP.bypass,
                    )
                    offs = r2.tile([P, 4], F32, name=f"offs{r}")
                    nc.vector.tensor_tensor(offs[:], btot_i[:], mm_sb[:, 0:4], op=OP.subtract)
                    nc.vector.tensor_tensor(offs[:], offs[:], mm_sb[:, 4:8], op=OP.add)
                    if r == 1:
                        nc.vector.tensor_scalar(offs[:], offs[:], base1[:], None, op0=OP.add)
                    cs = r2.tile([P, 4, P], F32, name=f"cs{r}")
                    for b in range(4):
                        nc.vector.tensor_scalar(
                            cs[:, b, :], ic[:, b, :], offs[:, b:b + 1], None, op0=OP.add
                        )
                    if r == 0:
                        # rank-1 base: min(total rank-0 assigned per expert, C);
                        # mm_sb[:, 0:4] holds per-expert block totals (Msame).
                        n0 = r2.tile([P, 1], F32)
                        nc.vector.tensor_reduce(n0[:], mm_sb[:, 0:4], axis=mybir.AxisListType.X, op=OP.add)
                        base1 = r2.tile([P, 1], F32)
                        nc.vector.tensor_scalar(base1[:], n0[:], float(C), None, op0=OP.min)
                    keep = r2.tile([P, 4, P], F32, name=f"keep{r}")
                    nc.vector.tensor_scalar(keep[:], cs[:], float(C), None, op0=OP.is_le)
                    kk = r2.tile([P, 4, P], F32, name=f"kk{r}")
                    nc.vector.tensor_tensor(kk[:], keep[:], ohT[:], op=OP.mult)
                    ksl = r2.tile([P, 4, P], F32, name=f"ksl{r}")
                    nc.vector.tensor_scalar(ksl[:], cs[:], -1.0, None, op0=OP.add)
                    nc.vector.tensor_tensor(ksl[:], ksl[:], kk[:], op=OP.mult)
                    # transpose back to token-major (two 1-bank psum tiles)
                    bk1 = bkps.tile([P, 4 * P], F32, space="PSUM", name="bk1")
                    bk2 = bkps.tile([P, 4 * P], F32, space="PSUM", name="bk2")
                    for b in range(4):
                        nc.tensor.matmul(
                            bk1[:, b * P:(b + 1) * P], ksl[:, b, :], ident[:],
                            is_transpose=True, start=(b == 0), stop=(b == 3),
                        )
                    for b in range(4):
                        nc.tensor.matmul(
                            bk2[:, b * P:(b + 1) * P], kk[:, b, :], ident[:],
                            is_transpose=True, start=(b == 0), stop=(b == 3),
                        )
                    cT = r2.tile([P, NT, E], F32, name=f"cT{r}")
                    nc.vector.tensor_copy(cT[:], bk1[:].rearrange("p (a e) -> p a e", e=E))
                    kTr = r2.tile([P, NT, E], F32, name=f"kTr{r}")
                    nc.vector.tensor_copy(kTr[:], bk2[:].rearrange("p (a e) -> p a e", e=E))
                    csm1T.append(cT)
                    kT.append(kTr)

                # ---- home-side indices (token-major, all tiles) ----
                esel_b = esel_sb[:].rearrange("p e -> p () e").to_broadcast([P, NT, E])
                erow_b = erow_sb[:].rearrange("p e -> p () e").to_broadcast([P, NT, E])
                tmp3 = r2.tile([P, NT, E], F32)
                gs = []
                cwf = []
                for r in range(2):
                    pos = r2.tile([P, NT], F32, name=f"pos{r}")
                    nc.vector.tensor_reduce(pos[:], csm1T[r][:], axis=mybir.AxisListType.X, op=OP.add)
                    keep_s = r2.tile([P, NT], F32, name=f"ks{r}")
                    nc.vector.tensor_reduce(keep_s[:], kT[r][:], axis=mybir.AxisListType.X, op=OP.max)
                    nc.vector.tensor_tensor(tmp3[:], ohs[r][:], erow_b, op=OP.mult)
                    es = r2.tile([P, NT], F32, name=f"es{r}")
                    nc.vector.tensor_reduce(es[:], tmp3[:], axis=mybir.AxisListType.X, op=OP.max)
                    # AG row index: 2048*(s//256) + 256*e + s%256
                    q = _ge_sum(nc, r2, pos, [256.0 * k for k in range(1, 8)], f"q{r}")
                    g = r2.tile([P, NT], F32, name=f"g{r}")
                    rem = r2.tile([P, NT], F32, name=f"rm{r}")
                    nc.vector.tensor_scalar(rem[:], q[:], -256.0, None, op0=OP.mult)
                    nc.vector.tensor_tensor(rem[:], rem[:], pos[:], op=OP.add)
                    nc.vector.tensor_scalar(g[:], q[:], 2048.0, None, op0=OP.mult)
                    t2 = r2.tile([P, NT], F32, name=f"t2{r}")
                    nc.vector.tensor_scalar(t2[:], es[:], 256.0, None, op0=OP.mult)
                    nc.vector.tensor_tensor(g[:], g[:], t2[:], op=OP.add)
                    nc.vector.tensor_tensor(g[:], g[:], rem[:], op=OP.add)
                    cw = r2.tile([P, NT], F32, name=f"cw{r}")
                    wsrc = wr1 if r == 0 else wr2
                    nc.vector.tensor_tensor(cw[:], wsrc[:], keep_s[:], op=OP.mult)
                    gs.append(g)
                    cwf.append(cw)
                    if debug:
                        nc.sync.dma_start([dbg_gs1, dbg_gs2][r][:], g[:])
                        nc.sync.dma_start([dbg_cw1, dbg_cw2][r][:], cw[:])
                        nc.sync.dma_start([dbg_pos1, dbg_pos2][r][:], pos[:])

                # select MY home block (hm64 mask + log-fold), build wrapped idx
                for r in range(2):
                    msk_g = r2.tile([P, NT], F32, name=f"mg{r}")
                    nc.vector.tensor_tensor(msk_g[:], gs[r][:], hm64_sb[:], op=OP.mult)
                    msk_c = r2.tile([P, NT], F32, name=f"mc{r}")
                    nc.vector.tensor_tensor(msk_c[:], cwf[r][:], hm64_sb[:], op=OP.mult)
                    for half in (32, 16, 8):
                        nc.vector.tensor_tensor(
                            msk_g[:, 0:half], msk_g[:, 0:half], msk_g[:, half:2 * half], op=OP.add
                        )
                        nc.vector.tensor_tensor(
                            msk_c[:, 0:half], msk_c[:, 0:half], msk_c[:, half:2 * half], op=OP.add
                        )
                    nc.vector.tensor_copy([cw1_my, cw2_my][r][:], msk_c[:, 0:E])
                    gi = r2.tile([P, E], I16, name=f"gi{r}")
                    nc.vector.tensor_copy(gi[:], msk_g[:, 0:E])
                    sh16 = [(i + 16) % 32 for i in range(32)]
                    gish = r2.tile([P, E], I16, name=f"gish{r}")
                    nc.vector.stream_shuffle(gish[:], gi[:], sh16)
                    idxh = [idx_h1, idx_h2][r]
                    idxh_v = idxh[0:16, :].rearrange("r (c q) -> r c q", q=8)
                    for qq in range(8):
                        src = gi if qq % 2 == 0 else gish
                        nc.vector.tensor_copy(
                            idxh_v[:, :, qq],
                            src[(qq // 2) * 32:(qq // 2) * 32 + 16, :],
                        )
                    for k in range(1, 8):
                        nc.sync.dma_start(idxh[16 * k:16 * (k + 1), :], idxh[0:16, :])

                # ---- expert-side slot->token map (my expert) ----
                ksl_e = r2.tile([P, NT], F32)
                k_e = r2.tile([P, NT], F32)
                acc = r2.tile([P, NT], F32)
                for r in range(2):
                    nc.vector.tensor_tensor(tmp3[:], csm1T[r][:], esel_b, op=OP.mult)
                    nc.vector.tensor_reduce(
                        (acc if r else ksl_e)[:], tmp3[:], axis=mybir.AxisListType.X, op=OP.add
                    )
                    if r:
                        nc.vector.tensor_tensor(ksl_e[:], ksl_e[:], acc[:], op=OP.add)
                    nc.vector.tensor_tensor(tmp3[:], kT[r][:], esel_b, op=OP.mult)
                    nc.vector.tensor_reduce(
                        (acc if r else k_e)[:], tmp3[:], axis=mybir.AxisListType.X, op=OP.max
                    )
                    if r:
                        nc.vector.tensor_tensor(k_e[:], k_e[:], acc[:], op=OP.max)
                # chi = s//128 in [0,16), remc = s%128 via two-level cascade
                q8 = _ge_sum(nc, r2, ksl_e, [512.0, 1024.0, 1536.0], "q8")
                s1 = r2.tile([P, NT], F32)
                nc.vector.tensor_scalar(s1[:], q8[:], -512.0, None, op0=OP.mult)
                nc.vector.tensor_tensor(s1[:], s1[:], ksl_e[:], op=OP.add)
                c3 = _ge_sum(nc, r2, s1, [128.0, 256.0, 384.0], "c3")
                chi = r2.tile([P, NT], F32)
                nc.vector.tensor_scalar(chi[:], q8[:], 4.0, None, op0=OP.mult)
                nc.vector.tensor_tensor(chi[:], chi[:], c3[:], op=OP.add)
                remc = r2.tile([P, NT], F32)
                nc.vector.tensor_scalar(remc[:], c3[:], -128.0, None, op0=OP.mult)
                nc.vector.tensor_tensor(remc[:], remc[:], s1[:], op=OP.add)
                ktlo = r2.tile([P, NT], F32)
                nc.vector.tensor_scalar(ktlo[:], k_e[:], tlo_sb[:], None, op0=OP.mult)
                kthi = r2.tile([P, NT], F32)
                nc.vector.tensor_tensor(kthi[:], k_e[:], thi_sb[:], op=OP.mult)

                o_all = r2.tile([P, NT, P], BF16)   # [s%128 == lo]
                nc.vector.tensor_tensor(
                    o_all[:], lo_sb[:],
                    remc[:].rearrange("p t -> p t ()").to_broadcast([P, NT, P]),
                    op=OP.is_equal,
                )
                v0 = r2.tile([P, NT, 16], F32)
                nc.vector.tensor_tensor(
                    v0[:], hi_sb[:],
                    chi[:].rearrange("p t -> p t ()").to_broadcast([P, NT, 16]),
                    op=OP.is_equal,
                )
                v_all = r2.tile([P, NT, 2, 16], BF16)
                nc.vector.tensor_tensor(
                    v_all[:, :, 0, :], v0[:],
                    ktlo[:].rearrange("p t -> p t ()").to_broadcast([P, NT, 16]),
                    op=OP.mult,
                )
                nc.vector.tensor_tensor(
                    v_all[:, :, 1, :], v0[:],
                    kthi[:].rearrange("p t -> p t ()").to_broadcast([P, NT, 16]),
                    op=OP.mult,
                )
                map_ps = mapps.tile([P, 2, 16], F32, space="PSUM")
                for j in range(NT):
                    nc.tensor.matmul(
                        map_ps[:].rearrange("p a b -> p (a b)"),
                        o_all[:, j, :],
                        v_all[:, j, :, :].rearrange("p a b -> p (a b)"),
                        start=(j == 0), stop=(j == NT - 1),
                    )
                map_sb = r2.tile([P, 2, 16], F32)
                nc.vector.tensor_copy(map_sb[:], map_ps[:])
                map_tok = r2.tile([P, 16], F32)
                nc.vector.tensor_scalar(map_tok[:], map_sb[:, 1, :], 64.0, None, op0=OP.mult)
                nc.vector.tensor_tensor(map_tok[:], map_tok[:], map_sb[:, 0, :], op=OP.add)
                if debug:
                    nc.sync.dma_start(dbg_map[:], map_tok[:])
                mi = r2.tile([P, 16], I16)
                nc.vector.tensor_copy(mi[:], map_tok[:])
                sh16 = [(i + 16) % 32 for i in range(32)]
                mish = r2.tile([P, 16], I16)
                nc.vector.stream_shuffle(mish[:], mi[:], sh16)
                idxx_v = idx_x[0:16, :].rearrange("r (h q) -> r h q", q=8)
                for qq in range(8):
                    src = mi if qq % 2 == 0 else mish
                    nc.vector.tensor_copy(
                        idxx_v[:, :, qq],
                        src[(qq // 2) * 32:(qq // 2) * 32 + 16, :],
                    )
                for k in range(1, 8):
                    nc.sync.dma_start(idx_x[16 * k:16 * (k + 1), :], idx_x[0:16, :])

            # ---------------- P3: FFN ----------------
            with (
                tc.tile_pool(name="w2pool", bufs=1) as w2pool,
                tc.tile_pool(name="xte", bufs=1) as xtep,
                tc.tile_pool(name="hall", bufs=1) as hallp,
                tc.tile_pool(name="ypool", bufs=1) as ypool,
                tc.tile_pool(name="hps", bufs=2, space="PSUM") as hps,
                tc.tile_pool(name="yps", bufs=1, space="PSUM") as yps,
            ):
                w2_sb = w2pool.tile([P, HC, D], BF16)
                for hc in range(HC):
                    nc.sync.dma_start(w2_sb[:, hc, :], w2_in[:, hc, :])
                h_all = hallp.tile([P, HC, SB], BF16)
                # all dispatch gathers up-front: gpsimd queue is in-order and
                # collective triggers block it, so gathers must precede them
                xTes = []
                for sb in range(NB):
                    xTe = xtep.tile([P, DC, SB], BF16, name=f"xTe{sb}")
                    nc.gpsimd.dma_gather(
                        out_ap=xTe[:],
                        in_ap=xb_all[:],
                        idxs_ap=idx_x[:, sb * (SB // 16):(sb + 1) * (SB // 16)],
                        num_idxs=SB, num_idxs_reg=SB, elem_size=D, transpose=True,
                    )
                    xTes.append(xTe)
                for sb in range(NB):
                    xTe = xTes[sb]
                    for hc in range(HC):
                        h_ps = hps.tile([P, SB], F32, space="PSUM", name="hps")
                        for dc in range(DC):
                            nc.tensor.matmul(
                                h_ps[:],
                                w1_sb[:, hc, dc, :],
                                xTe[:, dc, :],
                                start=(dc == 0), stop=(dc == DC - 1),
                            )
                        nc.scalar.activation(
                            h_all[:, hc, :], h_ps[:], AF.Gelu_apprx_tanh,
                            bias=b1_sb[:, hc:hc + 1],
                        )
                    for half in range(2):
                        y_ts = [
                            [yps.tile([P, 512], F32, space="PSUM", name=f"y{st}{dg}") for dg in range(2)]
                            for st in range(2)
                        ]
                        for hc in range(HC):
                            for st in range(2):
                                so = half * 256 + st * P
                                for dg in range(2):
                                    nc.tensor.matmul(
                                        y_ts[st][dg][:],
                                        h_all[:, hc, so:so + P],
                                        w2_sb[:, hc, dg * 512:(dg + 1) * 512],
                                        start=(hc == 0), stop=False,
                                    )
                        y_sb = ypool.tile([P, 2, D], BF16, name="ysb")
                        for st in range(2):
                            for dg in range(2):
                                nc.tensor.matmul(
                                    y_ts[st][dg][:], ones1[:],
                                    b2_sb[:, dg * 512:(dg + 1) * 512],
                                    start=False, stop=True,
                                )
                                nc.scalar.activation(
                                    y_sb[:, st, dg * 512:(dg + 1) * 512],
                                    y_ts[st][dg][:], AF.Copy,
                                )
                        r0 = sb * SB + half * 256
                        nc.sync.dma_start(
                            yag_in[r0:r0 + 256, :].rearrange("(s p) d -> p s d", p=P),
                            y_sb[:],
                        )
                        ch = 2 * sb + half
                        nc.gpsimd.collective_compute(
                            "AllGather", OP.bypass, replica_groups=[list(range(E))],
                            ins=[yag_in[ch * 256:(ch + 1) * 256, :].opt()],
                            outs=[yag_out[ch * 2048:(ch + 1) * 2048, :].opt()],
                        )

            # ---------------- P4: home combine (2 pipelined halves) -------
            NH = NTS // 2
            with tc.tile_pool(name="homep", bufs=2) as homep:
                for hh in range(2):
                    c0 = hh * NH
                    g1 = homep.tile([P, NH, D], BF16, name="g1")
                    nc.gpsimd.dma_gather(
                        out_ap=g1[:], in_ap=yag_out[:],
                        idxs_ap=idx_h1[:, c0 * 8:(c0 + NH) * 8],
                        num_idxs=NH * P, num_idxs_reg=NH * P, elem_size=D,
                    )
                    g2 = homep.tile([P, NH, D], BF16, name="g2")
                    nc.gpsimd.dma_gather(
                        out_ap=g2[:], in_ap=yag_out[:],
                        idxs_ap=idx_h2[:, c0 * 8:(c0 + NH) * 8],
                        num_idxs=NH * P, num_idxs_reg=NH * P, elem_size=D,
                    )
                    o1 = homep.tile([P, NH, D], F32, name="o1")
                    nc.vector.tensor_tensor(
                        o1[:], g1[:],
                        cw1_my[:, c0:c0 + NH].rearrange("p c -> p c ()").to_broadcast([P, NH, D]),
                        op=OP.mult,
                    )
                    o2 = homep.tile([P, NH, D], F32, name="o2")
                    nc.vector.tensor_tensor(
                        o2[:], g2[:],
                        cw2_my[:, c0:c0 + NH].rearrange("p c -> p c ()").to_broadcast([P, NH, D]),
                        op=OP.mult,
                    )
                    nc.vector.tensor_tensor(o1[:], o1[:], o2[:], op=OP.add)
                    nc.sync.dma_start(
                        out_sl[c0 * P:(c0 + NH) * P, :].rearrange("(a p) d -> p a d", p=P),
                        o1[:],
                    )

    nc.compile()
    return nc


_NC_CACHE = {}


def _get_nc(debug=False):
    key = f"nc{debug}"
    if key not in _NC_CACHE:
        _NC_CACHE[key] = build_moe(debug)
    return _NC_CACHE[key]


def make_inputs(x, Wg, W1, b1, W2, b2):
    """Host-side sharding: per-core input maps (data-independent prep only)."""
    bf = ml_dtypes.bfloat16
    x = np.ascontiguousarray(np.asarray(x, dtype=np.float32).reshape(T, D))
    wg = np.ascontiguousarray(
        np.asarray(Wg, dtype=np.float32).reshape(DC, P, E).transpose(1, 0, 2)
    )
    p = np.arange(P)
    tm = p // E
    ee = p % E
    msame = (ee[:, None] == ee[None, :]).astype(np.float32)        # [p', p]
    mlow = (msame * (tm[:, None] < tm[None, :])).astype(np.float32)
    erow = np.tile(np.arange(E, dtype=np.float32), (P, 1))
    loall = np.tile(np.arange(P, dtype=np.float32), (P, NT, 1)).astype(bf)
    hiall = np.tile(np.arange(16, dtype=np.float32), (P, NT, 1)).astype(np.float32)
    j = np.arange(NT)
    thi = (j[None, :] * 2 + (p // 64)[:, None]).astype(np.float32)  # (j*128+p)//64
    tlo = (p % 64).astype(np.float32).reshape(P, 1)

    in_maps = []
    for e in range(E):
        w1s = np.ascontiguousarray(
            np.asarray(W1[e], dtype=np.float32)
            .reshape(DC, P, HC, P).transpose(1, 2, 0, 3).astype(bf)
        )
        w2s = np.ascontiguousarray(
            np.asarray(W2[e], dtype=np.float32).reshape(HC, P, D).transpose(1, 0, 2).astype(bf)
        )
        b1s = np.ascontiguousarray(np.asarray(b1[e], dtype=np.float32).reshape(HC, P).T)
        b2r = np.asarray(b2[e], dtype=np.float32).reshape(1, D).astype(bf)
        esel = np.zeros((P, E), dtype=np.float32)
        esel[:, e] = 1.0
        hm64 = np.zeros((P, NT), dtype=np.float32)
        hm64[:, e * NTS:(e + 1) * NTS] = 1.0
        in_maps.append({
            "xs": np.ascontiguousarray(x[e * TS:(e + 1) * TS]),
            "wg": wg, "w1s": w1s, "w2s": w2s, "b1s": b1s, "b2r": b2r,
            "esel": esel, "hm64": hm64, "erow": erow,
            "msame": msame, "mlow": mlow, "loall": loall, "hiall": hiall,
            "thi": thi, "tlo": tlo,
        })
    return in_maps


def kernel(x, Wg, W1, b1, W2, b2):
    nc = _get_nc()
    in_maps = make_inputs(x, Wg, W1, b1, W2, b2)
    res = run_bass_kernel_spmd(nc, in_maps, list(range(E)))
    out = np.concatenate([res.results[e]["out_slice"] for e in range(E)], axis=0)
    return out.reshape(B, S, D).astype(np.float32)
